# revision 24
# baseline (speedup 1.0000x reference)
"""Trainium2 Bass kernel for a 2-layer GAT (CGATNet) over 100k nodes / 3.2M edges.

Strategy (8 NeuronCores):
  - Edges are sharded by DESTINATION-node range: core c owns dst in
    [c*SHARD, (c+1)*SHARD). Each core produces final output rows for its
    range -> no collective needed for the aggregation itself.
  - Node-level features (h = x@W plus folded attention logits al/ar) are
    computed sharded and AllGathered as a bf16 "gather table" with
    256B-aligned rows so dma_gather can fetch h[src] per edge.
  - Per 128-dst-node chunk: gather source rows, build one-hot S[e,d] via
    tensor_scalar(is_equal) against an iota row, broadcast ar[dst] to edges
    with a PE matmul (lhsT = S^T), compute p = max(exp(z), exp(0.2*z))
    (== exp(leaky_relu(z)) exactly, by monotonicity), scale messages
    in-place with a broadcast-AP multiply, and segment-sum via
    PSUM-accumulated matmuls out_chunk += S^T @ [p*h | p].
  - Epilogue per chunk: divide by the summed p (softmax denominator), apply
    bias + ELU, and immediately run the layer-2 node matmul for those rows.

Self-contained: only needs numpy/ml_dtypes plus the concourse runtime at
/opt/trn_rl_repo (the environment's Bass installation).
"""

import os
import sys

sys.path.insert(0, "/opt/trn_rl_repo")

import numpy as np
import ml_dtypes

from concourse import bass, bacc, mybir, tile
from concourse import bass_utils

BF16 = ml_dtypes.bfloat16

# ----------------------------------------------------------------------------
# configuration
# ----------------------------------------------------------------------------

def full_cfg():
    return dict(
        N=100000, F_IN=128, HID=16, HEADS=8, NCLS=40, NEG=0.2,
        NCORES=8, CHUNKS=98, BUCKET_ROWS=32768, GATHER="q7", GCAP=100000, GSP=False,
        NQ=4, AGSL=int(os.environ.get("GAT_AGSL", "0") or 0),
    )


def pd_cfg():
    return dict(
        N=100000, F_IN=128, HID=16, HEADS=8, NCLS=40, NEG=0.2,
        NCORES=8, CHUNKS=98, BUCKET_ROWS=25088, NQ=4,
        SCHEME="pd64", DCH=64, ARLO=True,
    )


def derive(cfg):
    c = dict(cfg)
    c["SHARD"] = c["CHUNKS"] * 128
    c["NPAD"] = c["NCORES"] * c["SHARD"]
    c["NBUCK"] = -(-c["NPAD"] // c["BUCKET_ROWS"])
    if c.get("SCHEME") == "pd64":
        c["ECH"] = c["SHARD"] // c["DCH"]
    # L1 table row: [h (128) | pslot (8) | al_f32 (16) | ar_f32 (16) | pad] bf16
    c["ROW1"] = 256
    c["H1"] = c["HEADS"] * c["HID"]          # 128
    c["MSG1"] = c["H1"] + c["HEADS"]          # 136
    c["ALF1"] = (c["H1"] + 8) // 2            # f32 col 68
    c["ARF1"] = c["ALF1"] + 8                 # f32 col 76
    # L2 table row: [h2 (320) | pslot (8) | al_f32 (16) | ar_f32 (16) | pad] bf16
    c["ROW2"] = 384
    c["H2"] = c["HEADS"] * c["NCLS"]          # 320
    c["MSG2"] = c["H2"] + c["HEADS"]          # 328
    c["ALF2"] = (c["H2"] + 8) // 2            # f32 col 164
    c["ARF2"] = c["ALF2"] + 8                 # f32 col 172
    return c


# ----------------------------------------------------------------------------
# host-side weight preparation
# ----------------------------------------------------------------------------

def prep_weights(cfg, x, W1, a_src1, a_dst1, b1, W2, a_src2, a_dst2, b2):
    c = cfg
    H, HID, NCLS = c["HEADS"], c["HID"], c["NCLS"]
    F = c["F_IN"]

    x = np.asarray(x, np.float32)
    W1 = np.asarray(W1, np.float32)
    W2 = np.asarray(W2, np.float32)
    a_src1 = np.asarray(a_src1, np.float32); a_dst1 = np.asarray(a_dst1, np.float32)
    a_src2 = np.asarray(a_src2, np.float32); a_dst2 = np.asarray(a_dst2, np.float32)
    b1 = np.asarray(b1, np.float32); b2 = np.asarray(b2, np.float32)

    # layer1: channel-major column permutation: col c*H+h <- original h*HID+c
    cm1 = np.array([h * HID + cc for cc in range(HID) for h in range(H)])
    W1cm = W1[:, cm1]                                   # [F, 128]
    Wal1 = np.einsum("fhc,hc->fh", W1.reshape(F, H, HID), a_src1)
    War1 = np.einsum("fhc,hc->fh", W1.reshape(F, H, HID), a_dst1)
    W1cat = np.concatenate([W1cm, Wal1, War1], axis=1)  # [F, 144]

    # layer2 rows must be permuted to match z1's channel-major feature order
    cm2 = np.array([h * NCLS + cc for cc in range(NCLS) for h in range(H)])
    W2cm = W2[:, cm2]                                   # [128, 320]
    Wal2 = np.einsum("fhc,hc->fh", W2.reshape(c["H1"], H, NCLS), a_src2)
    War2 = np.einsum("fhc,hc->fh", W2.reshape(c["H1"], H, NCLS), a_dst2)
    W2cat = np.concatenate([W2cm, Wal2, War2], axis=1)[cm1, :]  # [128, 336]

    b1cm = b1[cm1]

    xT = np.zeros((F, c["NPAD"]), np.float32)
    xT[:, : x.shape[0]] = x.T
    return dict(
        xT=xT.astype(BF16),
        W1cat=W1cat.astype(BF16),
        W2cat=W2cat.astype(BF16),
        b1rep=np.tile(b1cm, (128, 1)).astype(np.float32),
        b2rep=np.tile(b2, (128, 1)).astype(np.float32),
        iota=np.tile(np.arange(128, dtype=np.float32), (128, 1)).astype(BF16),
        ident=np.eye(128, dtype=np.float32).astype(BF16),
        iota_rep=None,  # filled in run() once NBmax is known
    )


# ----------------------------------------------------------------------------
# host-side edge preparation
# ----------------------------------------------------------------------------

class EdgeMeta:
    pass


def prep_edges_indirect(cfg, src, dst):
    """No-bucket variant: int32 row offsets for indirect_dma_start.
    Streams: offs [128, BCOLS] int32 (src row ids), drel [128, BCOLS] f32."""
    c = cfg
    NC, CH, SHARD = c["NCORES"], c["CHUNKS"], c["SHARD"]
    src = np.asarray(src, np.int64)
    dst = np.asarray(dst, np.int64)
    core = dst // SHARD
    chunk = (dst % SHARD) // 128
    rel = (dst % 128).astype(np.float32)
    key = (core * CH + chunk).astype(np.int64)
    order = np.argsort(key, kind="stable")
    s_s = src[order]
    rel_s = rel[order]
    counts = np.bincount(key[order], minlength=NC * CH).reshape(NC, CH)
    offs = np.zeros(NC * CH + 1, np.int64)
    np.cumsum(counts.reshape(-1), out=offs[1:])

    L = counts.max(axis=0)
    L = np.maximum(((L + 127) // 128) * 128, 128)
    NB_k = (L // 128).astype(int)
    assert NB_k.max() * c["HEADS"] <= 512, f"zp psum overflow: NB={NB_k.max()}"

    meta = EdgeMeta()
    meta.NBmax = int(NB_k.max())
    meta.chunks = []
    bcol = 0
    for k in range(CH):
        meta.chunks.append(dict(NB=int(NB_k[k]), bcol=bcol))
        bcol += int(NB_k[k])
    meta.BCOLS = bcol

    streams = []
    for cc in range(NC):
        off_cols = []
        drel_cols = []
        for k in range(CH):
            Lk = int(L[k])
            o0 = offs[cc * CH + k]
            o1 = offs[cc * CH + k + 1]
            n = int(o1 - o0)
            ids = np.zeros(Lk, np.int32)
            relv = np.full(Lk, -1.0, np.float32)
            if n > 0:
                ids[:n] = s_s[o0:o1]
                relv[:n] = rel_s[o0:o1]
            off_cols.append(ids.reshape(-1, 128).T)
            drel_cols.append(relv.reshape(-1, 128).T.astype(BF16))
        streams.append(dict(
            offs=np.ascontiguousarray(np.concatenate(off_cols, axis=1)),
            drel=np.ascontiguousarray(np.concatenate(drel_cols, axis=1)),
        ))
    assert streams[0]["offs"].shape[1] == meta.BCOLS
    return meta, streams


def prep_edges(cfg, src, dst):
    """Sort/bucket edges; build per-core idx + dst_rel streams with padding
    that is uniform across cores (the SPMD program is shared)."""
    if cfg.get("GATHER") == "indirect":
        return prep_edges_indirect(cfg, src, dst)
    c = cfg
    NC, CH, BR, NBUCK, SHARD = c["NCORES"], c["CHUNKS"], c["BUCKET_ROWS"], c["NBUCK"], c["SHARD"]

    src = np.asarray(src, np.int64)
    dst = np.asarray(dst, np.int64)

    core = dst // SHARD
    chunk = (dst % SHARD) // 128
    rel = (dst % 128).astype(np.float32)
    buck = src // BR
    key = ((core * CH + chunk) * NBUCK + buck).astype(np.int64)
    order = np.argsort(key, kind="stable")
    s_s = src[order]
    rel_s = rel[order]
    key_s = key[order]

    counts = np.bincount(key_s, minlength=NC * CH * NBUCK).reshape(NC, CH, NBUCK)
    offs = np.zeros(NC * CH * NBUCK + 1, np.int64)
    np.cumsum(counts.reshape(-1), out=offs[1:])

    # uniform (max-over-cores) padded sizes per (chunk, bucket)
    L = counts.max(axis=0)                     # [CH, NBUCK]
    L = ((L + 127) // 128) * 128
    empty = L.sum(axis=1) == 0
    L[empty, 0] = 128

    NB_k = (L.sum(axis=1) // 128).astype(int)  # batches per chunk
    assert NB_k.max() * c["HEADS"] <= 512, f"zp psum overflow: NB={NB_k.max()}"

    meta = EdgeMeta()
    meta.NBmax = int(NB_k.max())
    meta.chunks = []
    icol = 0
    bcol = 0
    for k in range(CH):
        buckets = []
        io_local = 0
        for b in range(NBUCK):
            Lkb = int(L[k, b])
            if Lkb == 0:
                continue
            buckets.append(dict(b=b, L=Lkb, icol=icol, io_local=io_local))
            icol += Lkb // 16
            io_local += Lkb // 16
        meta.chunks.append(dict(NB=int(NB_k[k]), bcol=bcol, buckets=buckets,
                                icol0=buckets[0]["icol"], icols=io_local))
        bcol += int(NB_k[k])
    meta.ICOLS = icol
    meta.BCOLS = bcol
    meta.ckmax = max(ch["icols"] for ch in meta.chunks)

    # build per-core streams
    streams = []
    for cc in range(NC):
        idx_blocks = []
        drel_cols = []
        for k in range(CH):
            chunk_rel = []
            for binfo in meta.chunks[k]["buckets"]:
                b = binfo["b"]; Lkb = binfo["L"]
                o0 = offs[(cc * CH + k) * NBUCK + b]
                o1 = offs[(cc * CH + k) * NBUCK + b + 1]
                n = int(o1 - o0)
                idx16 = np.zeros(Lkb, np.int16)
                relv = np.full(Lkb, -1.0, np.float32)
                if n > 0:
                    idx16[:n] = (s_s[o0:o1] - b * BR).astype(np.int16)
                    relv[:n] = rel_s[o0:o1]
                wrapped = idx16.reshape(-1, 16).T          # [16, L/16]
                idx_blocks.append(np.tile(wrapped, (8, 1)))  # [128, L/16]
                chunk_rel.append(relv)
            allrel = np.concatenate(chunk_rel)
            drel_cols.append(allrel.reshape(-1, 128).T.astype(BF16))  # [128, NB_k]
        streams.append(dict(
            idx=np.concatenate(idx_blocks, axis=1),
            drel=np.concatenate(drel_cols, axis=1),
        ))
    assert streams[0]["idx"].shape[1] == meta.ICOLS
    assert streams[0]["drel"].shape[1] == meta.BCOLS
    return meta, streams


# ----------------------------------------------------------------------------
# kernel builder
# ----------------------------------------------------------------------------

def _ap(base, dims, extra_offset=0):
    return bass.AP(base.tensor, base.offset + extra_offset, dims)


def build_kernel(cfg, meta):
    c = cfg
    CH, NBUCK, BR = c["CHUNKS"], c["NBUCK"], c["BUCKET_ROWS"]
    SHARD, NPAD, NC = c["SHARD"], c["NPAD"], c["NCORES"]
    H = c["HEADS"]
    NEG = c["NEG"]
    bf = mybir.dt.bfloat16
    f32 = mybir.dt.float32
    EPS = 1e-30
    Copy = mybir.ActivationFunctionType.Copy
    Exp = mybir.ActivationFunctionType.Exp

    NQ = c.get("NQ", 1)
    nc = bacc.Bacc("TRN2", target_bir_lowering=False, debug=False, num_devices=NC,
                   num_swdge_queues=NQ)
    qctr = [0]

    def next_q():
        q = qctr[0] % NQ
        qctr[0] += 1
        return q

    xT = nc.dram_tensor("xT", [128, SHARD], bf, kind="ExternalInput")
    W1c = nc.dram_tensor("W1c", [128, 144], bf, kind="ExternalInput")
    W2c = nc.dram_tensor("W2c", [128, c["H2"] + 16], bf, kind="ExternalInput")
    iota = nc.dram_tensor("iota", [128, 128], bf, kind="ExternalInput")
    ident = nc.dram_tensor("ident", [128, 128], bf, kind="ExternalInput")
    b1r = nc.dram_tensor("b1r", [128, 128], f32, kind="ExternalInput")
    b2r = nc.dram_tensor("b2r", [128, c["NCLS"]], f32, kind="ExternalInput")
    indirect = c.get("GATHER") == "indirect"
    if indirect:
        offsT = nc.dram_tensor("offs", [128, meta.BCOLS], mybir.dt.int32, kind="ExternalInput")
        drel = nc.dram_tensor("drel", [128, meta.BCOLS], bf, kind="ExternalInput")
    else:
        idxs = nc.dram_tensor("idxs", [128, meta.ICOLS], mybir.dt.int16, kind="ExternalInput")
        drel = nc.dram_tensor("drel", [128, meta.BCOLS], bf, kind="ExternalInput")
    iotar = nc.dram_tensor("iotar", [128, meta.NBmax * 128], bf, kind="ExternalInput")
    out = nc.dram_tensor("out", [SHARD, c["NCLS"]], f32, kind="ExternalOutput")

    NBmax = meta.NBmax

    with tile.TileContext(nc) as tc:
        with (
            tc.tile_pool(name="dram", bufs=1, space="DRAM") as dram,
            tc.tile_pool(name="const", bufs=1) as cp,
        ):
            AGSL = c.get("AGSL", 0)
            shr = {} if AGSL else {"addr_space": "Shared"}
            T1l = dram.tile([SHARD, c["ROW1"]], bf)
            T1 = dram.tile([NPAD, c["ROW1"]], bf, **shr)
            T2l = dram.tile([SHARD, c["ROW2"]], bf)
            T2 = dram.tile([NPAD, c["ROW2"]], bf, **shr)
            if AGSL:
                NSL = CH // AGSL
                sc1 = [dram.tile([NC * AGSL * 128, c["ROW1"]], bf, addr_space="Shared",
                                 name=f"sc1_{i}") for i in range(NSL)]
                sc2 = [dram.tile([NC * AGSL * 128, c["ROW2"]], bf, addr_space="Shared",
                                 name=f"sc2_{i}") for i in range(NSL)]

            iota_t = cp.tile([128, 128], bf, tag="iota")
            nc.sync.dma_start(out=iota_t[:], in_=iota[:])
            ident_t = cp.tile([128, 128], bf, tag="ident")
            nc.sync.dma_start(out=ident_t[:], in_=ident[:])
            b1_t = cp.tile([128, 128], f32, tag="b1")
            nc.sync.dma_start(out=b1_t[:], in_=b1r[:])
            b2_t = cp.tile([128, c["NCLS"]], f32, tag="b2")
            nc.sync.dma_start(out=b2_t[:], in_=b2r[:])
            W1_t = cp.tile([128, 144], bf, tag="W1")
            nc.sync.dma_start(out=W1_t[:], in_=W1c[:])
            W2_t = cp.tile([128, c["H2"] + 16], bf, tag="W2")
            nc.sync.dma_start(out=W2_t[:], in_=W2c[:])
            drel_t = cp.tile([128, meta.BCOLS], bf, tag="drel")
            nc.sync.dma_start(out=drel_t[:], in_=drel[:])
            iotar_t = cp.tile([128, meta.NBmax * 128], bf, tag="iotar")
            nc.sync.dma_start(out=iotar_t[:], in_=iotar[:])
            if indirect:
                offs_t = cp.tile([128, meta.BCOLS], mybir.dt.int32, tag="offs")
                nc.sync.dma_start(out=offs_t[:], in_=offsT[:])
            ar1_t = cp.tile([128, CH, 8], f32, tag="ar1")
            ar2_t = cp.tile([128, CH, 8], f32, tag="ar2")
            # bf16 hi/lo split of ar for bf16 zp matmuls (hi + lo == f32 value)
            ar1h_t = cp.tile([128, CH, 8], bf, tag="ar1h")
            ar1l_t = cp.tile([128, CH, 8], bf, tag="ar1l")
            ar2h_t = cp.tile([128, CH, 8], bf, tag="ar2h")
            ar2l_t = cp.tile([128, CH, 8], bf, tag="ar2l")

            # ---------------- phase A: L1 node matmuls ----------------
            with (
                tc.tile_pool(name="pa", bufs=2) as pa,
                tc.tile_pool(name="pap", bufs=2, space="PSUM") as pap,
            ):
                xT_t = pa.tile([128, SHARD], bf, tag="xT")
                nc.sync.dma_start(out=xT_t[:], in_=xT[:])
                SL = c.get("AGSL", 0)
                T1v = T1[:, :].rearrange("(cc s) r -> cc s r", cc=NC)
                for t in range(CH):
                    ps = pap.tile([128, 144], f32, tag="psA")
                    nc.tensor.matmul(ps[:], xT_t[:, t * 128:(t + 1) * 128], W1_t[:],
                                     start=True, stop=True)
                    row = pa.tile([128, c["MSG1"]], bf, tag="rowA")
                    nc.scalar.activation(out=row[:], in_=ps[:, 0:c["MSG1"]], func=Copy)
                    nc.sync.dma_start(out=T1l[t * 128:(t + 1) * 128, 0:c["MSG1"]], in_=row[:])
                    rowf = pa.tile([128, 16], f32, tag="rowAf")
                    nc.scalar.activation(out=rowf[:], in_=ps[:, c["H1"]:c["H1"] + 16], func=Copy)
                    t1l_f32 = T1l[t * 128:(t + 1) * 128, :].bitcast(f32)
                    nc.sync.dma_start(out=t1l_f32[:, c["ALF1"]:c["ALF1"] + 16], in_=rowf[:])
                    if SL and (t + 1) % SL == 0:
                        r0, r1 = (t + 1 - SL) * 128, (t + 1) * 128
                        sct = sc1[(t + 1) // SL - 1]
                        nc.gpsimd.collective_compute(
                            "AllGather", mybir.AluOpType.bypass,
                            replica_groups=[list(range(NC))],
                            ins=[T1l[r0:r1, :]], outs=[sct.opt()],
                        )
                        scv = sct[:, :].rearrange("(cc q) r -> cc q r", cc=NC)
                        nc.sync.dma_start(out=T1v[:, r0:r1, :], in_=scv)

            # ar1 preload (from local slice; rows (k*128+p), f32 cols ARF1..)
            t1l_all_f32 = T1l[:, :].bitcast(f32)
            ar1_src = t1l_all_f32.rearrange("(k p) r -> p k r", p=128)[:, :, c["ARF1"]:c["ARF1"] + 8]
            nc.sync.dma_start(out=ar1_t[:], in_=ar1_src)
            nc.scalar.activation(out=ar1h_t[:], in_=ar1_t[:], func=Copy)
            nc.vector.tensor_tensor(out=ar1l_t[:], in0=ar1_t[:], in1=ar1h_t[:],
                                    op=mybir.AluOpType.subtract)

            if not c.get("AGSL", 0):
                nc.gpsimd.collective_compute(
                    "AllGather", mybir.AluOpType.bypass,
                    replica_groups=[list(range(NC))],
                    ins=[T1l.opt()], outs=[T1.opt()],
                )

            def edge_chunk(k, lay, pool, psp):
                """Edge aggregation for chunk k of layer `lay`. Returns psum
                tile [128, MSG] holding [sum p*h | sum p] for the 128 dsts."""
                ROW = c["ROW1"] if lay == 1 else c["ROW2"]
                MSG = c["MSG1"] if lay == 1 else c["MSG2"]
                HW_ = c["H1"] if lay == 1 else c["H2"]
                ALF = c["ALF1"] if lay == 1 else c["ALF2"]
                CG = HW_ // 8
                tbl = T1 if lay == 1 else T2
                arh_t = ar1h_t if lay == 1 else ar2h_t
                arl_t = ar1l_t if lay == 1 else ar2l_t
                info = meta.chunks[k]
                NB = info["NB"]

                G = pool.tile([128, NBmax, ROW], bf, tag=f"G{lay}")
                if indirect:
                    nc.gpsimd.indirect_dma_start(
                        out=G[:, 0:NB, :],
                        out_offset=None,
                        in_=tbl[:, :],
                        in_offset=bass.IndirectOffsetOnAxis(
                            ap=offs_t[:, info["bcol"]:info["bcol"] + NB], axis=0),
                        queue_num=next_q(),
                    )
                else:
                    idx_t = pool.tile([128, meta.ckmax], mybir.dt.int16, tag=f"idx{lay}")
                    nc.sync.dma_start(out=idx_t[:, 0:info["icols"]],
                                      in_=idxs[:, info["icol0"]:info["icol0"] + info["icols"]])
                    gcap = c.get("GCAP", 1024)
                    slot = 0
                    for binfo in info["buckets"]:
                        b, L = binfo["b"], binfo["L"]
                        r0 = b * BR
                        r1 = min((b + 1) * BR, NPAD)
                        done = 0
                        while done < L:
                            Lp = min(gcap, L - done)
                            nb = Lp // 128
                            io = binfo["io_local"] + done // 16
                            nc.gpsimd.dma_gather(
                                out_ap=G[:, slot:slot + nb, :],
                                in_ap=tbl[r0:r1, :],
                                idxs_ap=idx_t[:, io:io + Lp // 16],
                                num_idxs=Lp, num_idxs_reg=Lp, elem_size=ROW,
                                single_packet=c.get("GSP", True),
                                queue_num=next_q(),
                            )
                            slot += nb
                            done += Lp

                Sall = pool.tile([128, NBmax, 128], bf, tag=f"S{lay}")
                zp = psp.tile([128, NBmax * 8], f32, tag="zp")
                db = drel_t[:, info["bcol"]:info["bcol"] + NB]
                nc.vector.tensor_tensor(
                    out=Sall[:, 0:NB, :],
                    in0=_ap(iotar_t[:, :], [iotar_t[:, :].ap[0], [128, NB], [1, 128]]),
                    in1=_ap(db, [db.ap[0], [1, NB], [0, 128]]),
                    op=mybir.AluOpType.is_equal,
                )
                ARLO = c.get("ARLO", True)
                GRP = c.get("GRP", 8)
                for j0 in range(0, NB, GRP):
                    g = min(GRP, NB - j0)
                    STg = psp.tile([128, GRP * 128], bf, tag="STg")
                    for j in range(j0, j0 + g):
                        nc.tensor.transpose(out=STg[:, (j - j0) * 128:(j - j0 + 1) * 128],
                                            in_=Sall[:, j, :], identity=ident_t[:])
                    STs = pool.tile([128, GRP * 128], bf, tag="STs")
                    nc.scalar.activation(out=STs[:, 0:g * 128], in_=STg[:, 0:g * 128], func=Copy)
                    for j in range(j0, j0 + g):
                        sl = STs[:, (j - j0) * 128:(j - j0 + 1) * 128]
                        nc.tensor.matmul(zp[:, j * 8:(j + 1) * 8], sl, arh_t[:, k, :],
                                         start=True, stop=not ARLO)
                        if ARLO:
                            nc.tensor.matmul(zp[:, j * 8:(j + 1) * 8], sl, arl_t[:, k, :],
                                             start=False, stop=True)

                # z = al[src] + ar[dst]  (both f32)
                Gf = G[:, :, :].bitcast(f32)   # [128, NBmax, ROW//2]
                al_view = Gf[:, 0:NB, ALF:ALF + 8]
                zs = pool.tile([128, NBmax, 8], f32, tag="zs")
                nc.vector.tensor_tensor(out=zs[:, 0:NB, :],
                                        in0=_ap(zp[:, 0:NB * 8], [zp[:, :].ap[0], [8, NB], [1, 8]]),
                                        in1=al_view, op=mybir.AluOpType.add)
                # p = max(exp(z), exp(0.2 z)) == exp(leaky_relu(z))
                e1 = pool.tile([128, NBmax, 8], f32, tag="e1")
                nc.scalar.activation(out=e1[:, 0:NB, :], in_=zs[:, 0:NB, :], func=Exp)
                e2 = pool.tile([128, NBmax, 8], f32, tag="e2")
                nc.scalar.activation(out=e2[:, 0:NB, :], in_=zs[:, 0:NB, :], func=Exp, scale=NEG)
                nc.vector.tensor_tensor(out=G[:, 0:NB, HW_:HW_ + 8], in0=e1[:, 0:NB, :],
                                        in1=e2[:, 0:NB, :], op=mybir.AluOpType.max)

                # messages in-place: G[:, :, :HW] *= p (broadcast over CG)
                gbase = G[:, :, :]
                part = gbase.ap[0]
                h_view = _ap(gbase, [part, [ROW, NB], [8, CG], [1, 8]])
                p_view = _ap(gbase, [part, [ROW, NB], [0, CG], [1, 8]], extra_offset=HW_)
                nc.vector.tensor_tensor(out=h_view, in0=h_view, in1=p_view,
                                        op=mybir.AluOpType.mult)

                pso = psp.tile([128, MSG], f32, tag="oc")
                for j in range(NB):
                    nc.tensor.matmul(pso[:], Sall[:, j, :], G[:, j, 0:MSG],
                                     start=(j == 0), stop=(j == NB - 1))
                return pso

            # ---------------- phase B + C: L1 edges + L2 node rows ----------------
            with (
                tc.tile_pool(name="pb", bufs=3) as pb,
                tc.tile_pool(name="pbp", bufs=2, space="PSUM") as pbp,
            ):
                for k in range(CH):
                    pso = edge_chunk(k, 1, pb, pbp)
                    # epilogue: out1 = pso[:, :128] / (denom+eps); z1 = elu(out1+b1)
                    rec = pb.tile([128, 8], f32, tag="rec")
                    nc.vector.tensor_scalar(out=rec[:], in0=pso[:, c["H1"]:c["H1"] + 8],
                                            scalar1=EPS, scalar2=None, op0=mybir.AluOpType.add)
                    nc.vector.reciprocal(out=rec[:], in_=rec[:])
                    t1v = pb.tile([128, 128], f32, tag="t1v")
                    rb = rec[:, :]
                    rec_b = _ap(rb, [rb.ap[0], [0, 16], [1, 8]])
                    nc.vector.tensor_tensor(
                        out=t1v[:, :], in0=pso[:, 0:c["H1"]],
                        in1=rec_b, op=mybir.AluOpType.mult)
                    nc.vector.tensor_tensor(out=t1v[:], in0=t1v[:], in1=b1_t[:],
                                            op=mybir.AluOpType.add)
                    # elu: z1 = max(t,0) + exp(min(t,0)) - 1
                    mn = pb.tile([128, 128], f32, tag="mn")
                    nc.vector.tensor_scalar(out=mn[:], in0=t1v[:], scalar1=0.0,
                                            scalar2=None, op0=mybir.AluOpType.min)
                    ex = pb.tile([128, 128], f32, tag="ex")
                    nc.scalar.activation(out=ex[:], in_=mn[:], func=Exp)
                    nc.vector.tensor_scalar(out=ex[:], in0=ex[:], scalar1=-1.0,
                                            scalar2=None, op0=mybir.AluOpType.add)
                    nc.vector.tensor_scalar(out=t1v[:], in0=t1v[:], scalar1=0.0,
                                            scalar2=None, op0=mybir.AluOpType.max)
                    z1 = pb.tile([128, 128], bf, tag="z1")
                    nc.vector.tensor_tensor(out=z1[:], in0=t1v[:], in1=ex[:],
                                            op=mybir.AluOpType.add)
                    # phase C: h2cat rows for these 128 nodes
                    z1Tp = pbp.tile([128, c.get("GRP", 8) * 128], bf, tag="STg")
                    nc.tensor.transpose(out=z1Tp[:, 0:128], in_=z1[:], identity=ident_t[:])
                    z1T = pb.tile([128, 128], bf, tag="z1T")
                    nc.scalar.activation(out=z1T[:], in_=z1Tp[:, 0:128], func=Copy)
                    h2p = pbp.tile([128, c["H2"] + 16], f32, tag="h2")
                    nc.tensor.matmul(h2p[:], z1T[:], W2_t[:], start=True, stop=True)
                    row2 = pb.tile([128, c["MSG2"]], bf, tag="row2")
                    nc.scalar.activation(out=row2[:], in_=h2p[:, 0:c["MSG2"]], func=Copy)
                    nc.sync.dma_start(out=T2l[k * 128:(k + 1) * 128, 0:c["MSG2"]], in_=row2[:])
                    row2f = pb.tile([128, 16], f32, tag="row2f")
                    nc.scalar.activation(out=row2f[:], in_=h2p[:, c["H2"]:c["H2"] + 16], func=Copy)
                    t2l_f32 = T2l[k * 128:(k + 1) * 128, :].bitcast(f32)
                    nc.sync.dma_start(out=t2l_f32[:, c["ALF2"]:c["ALF2"] + 16], in_=row2f[:])
                    SL = c.get("AGSL", 0)
                    if SL and (k + 1) % SL == 0:
                        r0, r1 = (k + 1 - SL) * 128, (k + 1) * 128
                        T2v = T2[:, :].rearrange("(cc s) r -> cc s r", cc=NC)
                        sck = sc2[(k + 1) // SL - 1]
                        nc.gpsimd.collective_compute(
                            "AllGather", mybir.AluOpType.bypass,
                            replica_groups=[list(range(NC))],
                            ins=[T2l[r0:r1, :]], outs=[sck.opt()],
                        )
                        scv2 = sck[:, :].rearrange("(cc q) r -> cc q r", cc=NC)
                        nc.sync.dma_start(out=T2v[:, r0:r1, :], in_=scv2)

            t2l_all_f32 = T2l[:, :].bitcast(f32)
            ar2_src = t2l_all_f32.rearrange("(k p) r -> p k r", p=128)[:, :, c["ARF2"]:c["ARF2"] + 8]
            nc.sync.dma_start(out=ar2_t[:], in_=ar2_src)
            nc.scalar.activation(out=ar2h_t[:], in_=ar2_t[:], func=Copy)
            nc.vector.tensor_tensor(out=ar2l_t[:], in0=ar2_t[:], in1=ar2h_t[:],
                                    op=mybir.AluOpType.subtract)

            if not c.get("AGSL", 0):
                nc.gpsimd.collective_compute(
                    "AllGather", mybir.AluOpType.bypass,
                    replica_groups=[list(range(NC))],
                    ins=[T2l.opt()], outs=[T2.opt()],
                )

            # ---------------- phase D: L2 edges + final output ----------------
            with (
                tc.tile_pool(name="pd", bufs=3) as pd,
                tc.tile_pool(name="pdp", bufs=2, space="PSUM") as pdp,
            ):
                for k in range(CH):
                    pso = edge_chunk(k, 2, pd, pdp)
                    rec = pd.tile([128, 8], f32, tag="rec2")
                    nc.vector.tensor_scalar(out=rec[:], in0=pso[:, c["H2"]:c["H2"] + 8],
                                            scalar1=EPS, scalar2=None, op0=mybir.AluOpType.add)
                    nc.vector.reciprocal(out=rec[:], in_=rec[:])
                    tv = pd.tile([128, c["H2"]], f32, tag="tv")
                    rb = rec[:, :]
                    rec_b = _ap(rb, [rb.ap[0], [0, c["NCLS"]], [1, 8]])
                    nc.vector.tensor_tensor(
                        out=tv[:, :], in0=pso[:, 0:c["H2"]],
                        in1=rec_b, op=mybir.AluOpType.mult)
                    red = pd.tile([128, c["NCLS"]], f32, tag="red")
                    nc.vector.tensor_reduce(
                        out=red[:],
                        in_=_ap(tv[:, :], [tv[:, :].ap[0], [8, c["NCLS"]], [1, 8]]),
                        axis=mybir.AxisListType.X, op=mybir.AluOpType.add)
                    nc.vector.tensor_scalar(out=red[:], in0=red[:], scalar1=1.0 / H,
                                            scalar2=None, op0=mybir.AluOpType.mult)
                    ot = pd.tile([128, c["NCLS"]], f32, tag="ot")
                    nc.vector.tensor_tensor(out=ot[:], in0=red[:], in1=b2_t[:],
                                            op=mybir.AluOpType.add)
                    nc.sync.dma_start(out=out[k * 128:(k + 1) * 128, :], in_=ot[:])

    nc.compile()
    return nc


# ----------------------------------------------------------------------------
# pd64 scheme: 64-dst chunks, one dst per partition per (chunk, bucket)
# ----------------------------------------------------------------------------

def _min_nb(cnts, nparts=128):
    n = max(1, int(np.ceil(cnts.sum() / nparts)))
    while np.ceil(cnts / n).sum() > nparts:
        n += 1
    return n


def prep_edges_pd(cfg, src, dst):
    c = cfg
    NC, CH, DCH = c["NCORES"], c["ECH"], c["DCH"]
    BR, NBUCK, SHARD = c["BUCKET_ROWS"], c["NBUCK"], c["SHARD"]

    src = np.asarray(src, np.int64)
    dst = np.asarray(dst, np.int64)
    core = dst // SHARD
    chunk = (dst % SHARD) // DCH
    rel = dst % DCH
    buck = src // BR

    key = (((core * CH + chunk) * NBUCK + buck) * DCH + rel)
    order = np.lexsort((src, key))
    s_s = src[order]
    key_s = key[order]
    counts = np.bincount(key_s, minlength=NC * CH * NBUCK * DCH).reshape(
        NC, CH, NBUCK, DCH)
    offs = np.zeros(NC * CH * NBUCK * DCH + 1, np.int64)
    np.cumsum(counts.reshape(-1), out=offs[1:])

    NBb = np.zeros((CH, NBUCK), int)
    for k in range(CH):
        for b in range(NBUCK):
            NBb[k, b] = max(_min_nb(counts[cc, k, b]) for cc in range(NC))

    meta = EdgeMeta()
    meta.chunks = []
    icol = 0
    bcol = 0
    for k in range(CH):
        segs = []
        slot = 0
        for b in range(NBUCK):
            nb = int(NBb[k, b])
            segs.append(dict(b=b, nb=nb, slot=slot, icol=icol))
            slot += nb
            icol += nb * 8
        meta.chunks.append(dict(NB=slot, bcol=bcol, segs=segs))
        bcol += slot
    meta.ICOLS = icol
    meta.BCOLS = bcol
    meta.NBmax = max(ch["NB"] for ch in meta.chunks)
    meta.ckicols = max(sum(s["nb"] * 8 for s in ch["segs"]) for ch in meta.chunks)

    streams = []
    for cc in range(NC):
        idx_cols = []
        mask_cols = []
        pdst_cols = []
        for k in range(CH):
            for seg in meta.chunks[k]["segs"]:
                b, nb = seg["b"], seg["nb"]
                cnt = counts[cc, k, b]
                part_rel = np.full(128, -1, np.int64)
                grid_src = np.zeros((128, nb), np.int64)
                valid = np.zeros((128, nb), bool)
                p = 0
                for r in range(DCH):
                    n = int(cnt[r])
                    if n == 0:
                        continue
                    o0 = offs[((cc * CH + k) * NBUCK + b) * DCH + r]
                    edges = s_s[o0:o0 + n] - b * BR
                    pos = 0
                    while pos < n:
                        take = min(nb, n - pos)
                        assert p < 128, f"pd pack overflow c{cc} k{k} b{b}"
                        part_rel[p] = r
                        grid_src[p, :take] = edges[pos:pos + take]
                        valid[p, :take] = True
                        pos += take
                        p += 1
                idx16 = grid_src.T.reshape(-1).astype(np.int16)
                wrapped = idx16.reshape(-1, 16).T
                idx_cols.append(np.tile(wrapped, (8, 1)))
                mask_cols.append(valid.astype(np.float32).astype(BF16))
                pdst_cols.append(part_rel.astype(np.float32).astype(BF16))
        streams.append(dict(
            idx=np.ascontiguousarray(np.concatenate(idx_cols, axis=1)),
            mask=np.ascontiguousarray(np.concatenate(mask_cols, axis=1)),
            pdst=np.ascontiguousarray(np.stack(pdst_cols, axis=1)),
        ))
    assert streams[0]["idx"].shape[1] == meta.ICOLS
    assert streams[0]["mask"].shape[1] == meta.BCOLS
    return meta, streams


def build_kernel_pd(cfg, meta):
    c = cfg
    PCH, ECH, NBUCK, BR = c["CHUNKS"], c["ECH"], c["NBUCK"], c["BUCKET_ROWS"]
    SHARD, NPAD, NC, DCH = c["SHARD"], c["NPAD"], c["NCORES"], c["DCH"]
    H = c["HEADS"]
    NEG = c["NEG"]
    ARLO = c.get("ARLO", True)
    bf = mybir.dt.bfloat16
    f32 = mybir.dt.float32
    EPS = 1e-30
    Copy = mybir.ActivationFunctionType.Copy
    Exp = mybir.ActivationFunctionType.Exp
    NBmax = meta.NBmax

    NQ = c.get("NQ", 1)
    nc = bacc.Bacc("TRN2", target_bir_lowering=False, debug=False, num_devices=NC,
                   num_swdge_queues=NQ)
    qctr = [0]

    def next_q():
        q = qctr[0] % NQ
        qctr[0] += 1
        return q

    xT = nc.dram_tensor("xT", [128, SHARD], bf, kind="ExternalInput")
    W1c = nc.dram_tensor("W1c", [128, 144], bf, kind="ExternalInput")
    W2c = nc.dram_tensor("W2c", [128, c["H2"] + 16], bf, kind="ExternalInput")
    iota = nc.dram_tensor("iota", [128, 128], bf, kind="ExternalInput")
    ident = nc.dram_tensor("ident", [128, 128], bf, kind="ExternalInput")
    b1r = nc.dram_tensor("b1r", [128, 128], f32, kind="ExternalInput")
    b2r = nc.dram_tensor("b2r", [128, c["NCLS"]], f32, kind="ExternalInput")
    idxs = nc.dram_tensor("idxs", [128, meta.ICOLS], mybir.dt.int16, kind="ExternalInput")
    maskS = nc.dram_tensor("maskS", [128, meta.BCOLS], bf, kind="ExternalInput")
    pdstS = nc.dram_tensor("pdstS", [128, ECH * NBUCK], bf, kind="ExternalInput")
    out = nc.dram_tensor("out", [SHARD, c["NCLS"]], f32, kind="ExternalOutput")

    with tile.TileContext(nc) as tc:
        with (
            tc.tile_pool(name="dram", bufs=1, space="DRAM") as dram,
            tc.tile_pool(name="const", bufs=1) as cp,
        ):
            T1l = dram.tile([SHARD, c["ROW1"]], bf)
            T1 = dram.tile([NPAD, c["ROW1"]], bf, addr_space="Shared")
            T2l = dram.tile([SHARD, c["ROW2"]], bf)
            T2 = dram.tile([NPAD, c["ROW2"]], bf, addr_space="Shared")

            iota_t = cp.tile([128, 128], bf, tag="iota")
            nc.sync.dma_start(out=iota_t[:], in_=iota[:])
            ident_t = cp.tile([128, 128], bf, tag="ident")
            nc.sync.dma_start(out=ident_t[:], in_=ident[:])
            b1_t = cp.tile([128, 128], f32, tag="b1")
            nc.sync.dma_start(out=b1_t[:], in_=b1r[:])
            b2_t = cp.tile([128, c["NCLS"]], f32, tag="b2")
            nc.sync.dma_start(out=b2_t[:], in_=b2r[:])
            W1_t = cp.tile([128, 144], bf, tag="W1")
            nc.sync.dma_start(out=W1_t[:], in_=W1c[:])
            W2_t = cp.tile([128, c["H2"] + 16], bf, tag="W2")
            nc.sync.dma_start(out=W2_t[:], in_=W2c[:])
            mask_t = cp.tile([128, meta.BCOLS], bf, tag="mask")
            nc.sync.dma_start(out=mask_t[:], in_=maskS[:])
            pdst_t = cp.tile([128, ECH * NBUCK], bf, tag="pdst")
            nc.sync.dma_start(out=pdst_t[:], in_=pdstS[:])
            ar1_t = cp.tile([64, ECH, 8], f32, tag="ar1")
            ar2_t = cp.tile([64, ECH, 8], f32, tag="ar2")
            ar1h_t = cp.tile([64, ECH, 8], bf, tag="ar1h")
            ar1l_t = cp.tile([64, ECH, 8], bf, tag="ar1l")
            ar2h_t = cp.tile([64, ECH, 8], bf, tag="ar2h")
            ar2l_t = cp.tile([64, ECH, 8], bf, tag="ar2l")

            # ---------------- phase A: L1 node matmuls ----------------
            with (
                tc.tile_pool(name="pa", bufs=2) as pa,
                tc.tile_pool(name="pap", bufs=2, space="PSUM") as pap,
            ):
                xT_t = pa.tile([128, SHARD], bf, tag="xT")
                nc.sync.dma_start(out=xT_t[:], in_=xT[:])
                for t in range(PCH):
                    ps = pap.tile([128, 144], f32, tag="psA")
                    nc.tensor.matmul(ps[:], xT_t[:, t * 128:(t + 1) * 128], W1_t[:],
                                     start=True, stop=True)
                    row = pa.tile([128, c["MSG1"]], bf, tag="rowA")
                    nc.scalar.activation(out=row[:], in_=ps[:, 0:c["MSG1"]], func=Copy)
                    nc.sync.dma_start(out=T1l[t * 128:(t + 1) * 128, 0:c["MSG1"]], in_=row[:])
                    rowf = pa.tile([128, 16], f32, tag="rowAf")
                    nc.scalar.activation(out=rowf[:], in_=ps[:, c["H1"]:c["H1"] + 16], func=Copy)
                    t1l_f32 = T1l[t * 128:(t + 1) * 128, :].bitcast(f32)
                    nc.sync.dma_start(out=t1l_f32[:, c["ALF1"]:c["ALF1"] + 16], in_=rowf[:])

            t1l_all_f32 = T1l[:, :].bitcast(f32)
            ar1_src = t1l_all_f32.rearrange("(k p) r -> p k r", p=DCH)[:, :, c["ARF1"]:c["ARF1"] + 8]
            nc.sync.dma_start(out=ar1_t[:], in_=ar1_src)
            nc.scalar.activation(out=ar1h_t[:], in_=ar1_t[:], func=Copy)
            nc.vector.tensor_tensor(out=ar1l_t[:], in0=ar1_t[:], in1=ar1h_t[:],
                                    op=mybir.AluOpType.subtract)

            nc.gpsimd.collective_compute(
                "AllGather", mybir.AluOpType.bypass,
                replica_groups=[list(range(NC))],
                ins=[T1l.opt()], outs=[T1.opt()],
            )

            def edge_chunk(k, lay, pool, psp, psp1):
                ROW = c["ROW1"] if lay == 1 else c["ROW2"]
                MSG = c["MSG1"] if lay == 1 else c["MSG2"]
                HW_ = c["H1"] if lay == 1 else c["H2"]
                ALF = c["ALF1"] if lay == 1 else c["ALF2"]
                CG = HW_ // 8
                tbl = T1 if lay == 1 else T2
                arh_t = ar1h_t if lay == 1 else ar2h_t
                arl_t = ar1l_t if lay == 1 else ar2l_t
                info = meta.chunks[k]
                NB = info["NB"]
                segs = info["segs"]
                icol0 = segs[0]["icol"]
                icols = sum(s["nb"] * 8 for s in segs)

                G = pool.tile([128, NBmax, ROW], bf, tag=f"G{lay}")
                idx_t = pool.tile([128, meta.ckicols], mybir.dt.int16, tag=f"idx{lay}")
                nc.sync.dma_start(out=idx_t[:, 0:icols],
                                  in_=idxs[:, icol0:icol0 + icols])
                for seg in segs:
                    b, nb, slot = seg["b"], seg["nb"], seg["slot"]
                    io = seg["icol"] - icol0
                    nc.gpsimd.dma_gather(
                        out_ap=G[:, slot:slot + nb, :],
                        in_ap=tbl[b * BR:(b + 1) * BR, :],
                        idxs_ap=idx_t[:, io:io + nb * 8],
                        num_idxs=nb * 128, num_idxs_reg=nb * 128, elem_size=ROW,
                        single_packet=False,
                        queue_num=next_q(),
                    )

                Gf = G[:, :, :].bitcast(f32)
                zs = pool.tile([128, NBmax, 8], f32, tag="zs")
                pms = []
                for si, seg in enumerate(segs):
                    b, nb, slot = seg["b"], seg["nb"], seg["slot"]
                    Pm = pool.tile([128, 64], bf, tag=f"Pm{si}")
                    pd = pdst_t[:, k * NBUCK + b:k * NBUCK + b + 1]
                    nc.vector.tensor_tensor(
                        out=Pm[:], in0=iota_t[:, 0:64],
                        in1=_ap(pd, [pd.ap[0], [0, 64]]),
                        op=mybir.AluOpType.is_equal)
                    PmTp = psp1.tile([64, 128], bf, tag="PmTp")
                    nc.tensor.transpose(out=PmTp[:], in_=Pm[:], identity=ident_t[:])
                    PmT = pool.tile([64, 128], bf, tag="PmT")
                    nc.scalar.activation(out=PmT[:], in_=PmTp[:], func=Copy)
                    arp_ps = psp1.tile([128, 8], f32, tag="arp")
                    nc.tensor.matmul(arp_ps[:], PmT[:], arh_t[0:64, k, :],
                                     start=True, stop=not ARLO)
                    if ARLO:
                        nc.tensor.matmul(arp_ps[:], PmT[:], arl_t[0:64, k, :],
                                         start=False, stop=True)
                    arp = pool.tile([128, 8], f32, tag=f"arp{si}")
                    nc.scalar.activation(out=arp[:], in_=arp_ps[:], func=Copy)
                    a_v = arp[:, :]
                    nc.vector.tensor_tensor(
                        out=zs[:, slot:slot + nb, :],
                        in0=Gf[:, slot:slot + nb, ALF:ALF + 8],
                        in1=_ap(a_v, [a_v.ap[0], [0, nb], [1, 8]]),
                        op=mybir.AluOpType.add)
                    pms.append(Pm)

                e1 = pool.tile([128, NBmax, 8], f32, tag="e1")
                nc.scalar.activation(out=e1[:, 0:NB, :], in_=zs[:, 0:NB, :], func=Exp)
                e2 = pool.tile([128, NBmax, 8], f32, tag="e2")
                nc.scalar.activation(out=e2[:, 0:NB, :], in_=zs[:, 0:NB, :], func=Exp, scale=NEG)
                pmx = pool.tile([128, NBmax, 8], f32, tag="pmx")
                nc.vector.tensor_tensor(out=pmx[:, 0:NB, :], in0=e1[:, 0:NB, :],
                                        in1=e2[:, 0:NB, :], op=mybir.AluOpType.max)
                msk = mask_t[:, info["bcol"]:info["bcol"] + NB]
                nc.vector.tensor_tensor(
                    out=G[:, 0:NB, HW_:HW_ + 8], in0=pmx[:, 0:NB, :],
                    in1=_ap(msk, [msk.ap[0], [1, NB], [0, 8]]),
                    op=mybir.AluOpType.mult)

                gbase = G[:, :, :]
                part = gbase.ap[0]
                h_view = _ap(gbase, [part, [ROW, NB], [8, CG], [1, 8]])
                p_view = _ap(gbase, [part, [ROW, NB], [0, CG], [1, 8]], extra_offset=HW_)
                nc.vector.tensor_tensor(out=h_view, in0=h_view, in1=p_view,
                                        op=mybir.AluOpType.mult)

                pso = psp.tile([64, MSG], f32, tag="oc")
                jg = 0
                for si, seg in enumerate(segs):
                    for j in range(seg["nb"]):
                        nc.tensor.matmul(pso[:], pms[si][:], G[:, seg["slot"] + j, 0:MSG],
                                         start=(jg == 0), stop=(jg == NB - 1))
                        jg += 1
                return pso

            # ---------------- phase B + C: L1 edges + L2 node rows ----------------
            with (
                tc.tile_pool(name="pb", bufs=2) as pb,
                tc.tile_pool(name="pbp", bufs=2, space="PSUM") as pbp,
                tc.tile_pool(name="pbp1", bufs=1, space="PSUM") as pbp1,
            ):
                for k in range(ECH):
                    pso = edge_chunk(k, 1, pb, pbp, pbp1)
                    rec = pb.tile([64, 8], f32, tag="rec")
                    nc.vector.tensor_scalar(out=rec[:], in0=pso[:, c["H1"]:c["H1"] + 8],
                                            scalar1=EPS, scalar2=None, op0=mybir.AluOpType.add)
                    nc.vector.reciprocal(out=rec[:], in_=rec[:])
                    t1v = pb.tile([64, 128], f32, tag="t1v")
                    rb = rec[:, :]
                    rec_b = _ap(rb, [rb.ap[0], [0, 16], [1, 8]])
                    nc.vector.tensor_tensor(
                        out=t1v[:, :], in0=pso[:, 0:c["H1"]],
                        in1=rec_b, op=mybir.AluOpType.mult)
                    nc.vector.tensor_tensor(out=t1v[:], in0=t1v[:], in1=b1_t[0:64, :],
                                            op=mybir.AluOpType.add)
                    mn = pb.tile([64, 128], f32, tag="mn")
                    nc.vector.tensor_scalar(out=mn[:], in0=t1v[:], scalar1=0.0,
                                            scalar2=None, op0=mybir.AluOpType.min)
                    ex = pb.tile([64, 128], f32, tag="ex")
                    nc.scalar.activation(out=ex[:], in_=mn[:], func=Exp)
                    nc.vector.tensor_scalar(out=ex[:], in0=ex[:], scalar1=-1.0,
                                            scalar2=None, op0=mybir.AluOpType.add)
                    nc.vector.tensor_scalar(out=t1v[:], in0=t1v[:], scalar1=0.0,
                                            scalar2=None, op0=mybir.AluOpType.max)
                    z1 = pb.tile([64, 128], bf, tag="z1")
                    nc.vector.tensor_tensor(out=z1[:], in0=t1v[:], in1=ex[:],
                                            op=mybir.AluOpType.add)
                    z1Tp = pbp1.tile([128, 64], bf, tag="zT")
                    nc.tensor.transpose(out=z1Tp[:], in_=z1[:], identity=ident_t[0:64, 0:64])
                    z1T = pb.tile([128, 64], bf, tag="z1T")
                    nc.scalar.activation(out=z1T[:], in_=z1Tp[:], func=Copy)
                    h2p = pbp1.tile([64, c["H2"] + 16], f32, tag="h2")
                    nc.tensor.matmul(h2p[:], z1T[:], W2_t[:], start=True, stop=True)
                    row2 = pb.tile([64, c["MSG2"]], bf, tag="row2")
                    nc.scalar.activation(out=row2[:], in_=h2p[:, 0:c["MSG2"]], func=Copy)
                    nc.sync.dma_start(out=T2l[k * DCH:(k + 1) * DCH, 0:c["MSG2"]], in_=row2[:])
                    row2f = pb.tile([64, 16], f32, tag="row2f")
                    nc.scalar.activation(out=row2f[:], in_=h2p[:, c["H2"]:c["H2"] + 16], func=Copy)
                    t2l_f32 = T2l[k * DCH:(k + 1) * DCH, :].bitcast(f32)
                    nc.sync.dma_start(out=t2l_f32[:, c["ALF2"]:c["ALF2"] + 16], in_=row2f[:])

            t2l_all_f32 = T2l[:, :].bitcast(f32)
            ar2_src = t2l_all_f32.rearrange("(k p) r -> p k r", p=DCH)[:, :, c["ARF2"]:c["ARF2"] + 8]
            nc.sync.dma_start(out=ar2_t[:], in_=ar2_src)
            nc.scalar.activation(out=ar2h_t[:], in_=ar2_t[:], func=Copy)
            nc.vector.tensor_tensor(out=ar2l_t[:], in0=ar2_t[:], in1=ar2h_t[:],
                                    op=mybir.AluOpType.subtract)

            nc.gpsimd.collective_compute(
                "AllGather", mybir.AluOpType.bypass,
                replica_groups=[list(range(NC))],
                ins=[T2l.opt()], outs=[T2.opt()],
            )

            # ---------------- phase D: L2 edges + final output ----------------
            with (
                tc.tile_pool(name="pd", bufs=2) as pd_,
                tc.tile_pool(name="pdp", bufs=2, space="PSUM") as pdp,
                tc.tile_pool(name="pdp1", bufs=1, space="PSUM") as pdp1,
            ):
                for k in range(ECH):
                    pso = edge_chunk(k, 2, pd_, pdp, pdp1)
                    rec = pd_.tile([64, 8], f32, tag="rec2")
                    nc.vector.tensor_scalar(out=rec[:], in0=pso[:, c["H2"]:c["H2"] + 8],
                                            scalar1=EPS, scalar2=None, op0=mybir.AluOpType.add)
                    nc.vector.reciprocal(out=rec[:], in_=rec[:])
                    tv = pd_.tile([64, c["H2"]], f32, tag="tv")
                    rb = rec[:, :]
                    rec_b = _ap(rb, [rb.ap[0], [0, c["NCLS"]], [1, 8]])
                    nc.vector.tensor_tensor(
                        out=tv[:, :], in0=pso[:, 0:c["H2"]],
                        in1=rec_b, op=mybir.AluOpType.mult)
                    red = pd_.tile([64, c["NCLS"]], f32, tag="red")
                    nc.vector.tensor_reduce(
                        out=red[:],
                        in_=_ap(tv[:, :], [tv[:, :].ap[0], [8, c["NCLS"]], [1, 8]]),
                        axis=mybir.AxisListType.X, op=mybir.AluOpType.add)
                    nc.vector.tensor_scalar(out=red[:], in0=red[:], scalar1=1.0 / H,
                                            scalar2=None, op0=mybir.AluOpType.mult)
                    ot = pd_.tile([64, c["NCLS"]], f32, tag="ot")
                    nc.vector.tensor_tensor(out=ot[:], in0=red[:], in1=b2_t[0:64, :],
                                            op=mybir.AluOpType.add)
                    nc.sync.dma_start(out=out[k * DCH:(k + 1) * DCH, :], in_=ot[:])

    nc.compile()
    return nc


def make_in_maps_pd(cfg, weights, meta, streams):
    c = cfg
    in_maps = []
    for cc in range(c["NCORES"]):
        in_maps.append(dict(
            xT=np.ascontiguousarray(weights["xT"][:, cc * c["SHARD"]:(cc + 1) * c["SHARD"]]),
            W1c=weights["W1cat"],
            W2c=weights["W2cat"],
            iota=weights["iota"],
            ident=weights["ident"],
            b1r=weights["b1rep"],
            b2r=weights["b2rep"],
            idxs=streams[cc]["idx"],
            maskS=streams[cc]["mask"],
            pdstS=streams[cc]["pdst"],
        ))
    return in_maps


# ----------------------------------------------------------------------------
# driver
# ----------------------------------------------------------------------------

def _install_ntff_shim():
    """Register the axon NTFF profiling hook (for trace=True runs)."""
    import types
    if "antenv.axon_hooks" not in sys.modules:
        try:
            import antenv  # noqa: F401
        except ImportError:
            pkg = types.ModuleType("antenv")
            pkg.__path__ = []
            sys.modules["antenv"] = pkg
        mod = types.ModuleType("antenv.axon_hooks")
        _hook = [None]
        mod.set_axon_ntff_profile_hook = lambda h: _hook.__setitem__(0, h)
        mod.get_axon_ntff_profile_hook = lambda: _hook[0]
        sys.modules["antenv"].axon_hooks = mod
        sys.modules["antenv.axon_hooks"] = mod
    try:
        from trn_agent_boot.trn_boot import _ntff_profile_via_ctypes
        from antenv.axon_hooks import set_axon_ntff_profile_hook
        set_axon_ntff_profile_hook(_ntff_profile_via_ctypes("/opt/axon/libaxon_pjrt.so"))
        bass_utils.upload_artifacts = lambda tmpdir: tmpdir
    except Exception:
        pass


def make_in_maps(cfg, weights, meta, streams):
    c = cfg
    indirect = cfg.get("GATHER") == "indirect"
    iota_rep = np.tile(np.tile(np.arange(128, dtype=np.float32), (128, 1)).astype(BF16),
                       (1, meta.NBmax))
    in_maps = []
    for cc in range(c["NCORES"]):
        in_maps.append(dict(
            xT=np.ascontiguousarray(weights["xT"][:, cc * c["SHARD"]:(cc + 1) * c["SHARD"]]),
            W1c=weights["W1cat"],
            W2c=weights["W2cat"],
            iota=weights["iota"],
            ident=weights["ident"],
            b1r=weights["b1rep"],
            b2r=weights["b2rep"],
            drel=streams[cc]["drel"],
            iotar=iota_rep,
            **({"offs": streams[cc]["offs"]} if indirect
               else {"idxs": streams[cc]["idx"]}),
        ))
    return in_maps


def run(cfg, inputs, trace=False):
    c = derive(cfg)
    x = np.asarray(inputs["x"])
    ei = np.asarray(inputs["edge_index"])
    weights = prep_weights(
        c, x, inputs["W1"], inputs["att_src1"], inputs["att_dst1"], inputs["b1"],
        inputs["W2"], inputs["att_src2"], inputs["att_dst2"], inputs["b2"])
    if c.get("SCHEME") == "pd64":
        meta, streams = prep_edges_pd(c, ei[0], ei[1])
        nc = build_kernel_pd(c, meta)
        in_maps = make_in_maps_pd(c, weights, meta, streams)
    else:
        meta, streams = prep_edges(c, ei[0], ei[1])
        nc = build_kernel(c, meta)
        in_maps = make_in_maps(c, weights, meta, streams)
    if trace:
        _install_ntff_shim()
    res = bass_utils.run_bass_kernel_spmd(
        nc, in_maps, core_ids=list(range(c["NCORES"])), trace=trace)
    outs = [res.results[cc]["out"] for cc in range(c["NCORES"])]
    full = np.concatenate(outs, axis=0)[: c["N"]]
    return full, res


def active_cfg():
    if os.environ.get("GAT_SCHEME") == "pd64":
        return pd_cfg()
    if os.environ.get("GAT_SCHEME") == "base":
        return full_cfg()
    return full_cfg()


def kernel(**inputs):
    out, _ = run(active_cfg(), inputs, trace=bool(os.environ.get("GAT_TRACE")))
    return out



# revision 27
# speedup vs baseline: 1.1299x; 1.1299x over previous
"""Trainium2 Bass kernel for a 2-layer GAT (CGATNet) over 100k nodes / 3.2M edges.

Strategy (8 NeuronCores):
  - Edges are sharded by DESTINATION-node range: core c owns dst in
    [c*SHARD, (c+1)*SHARD). Each core produces final output rows for its
    range -> no collective needed for the aggregation itself.
  - Node-level features (h = x@W plus folded attention logits al/ar) are
    computed sharded and AllGathered as a bf16 "gather table" with
    256B-aligned rows so dma_gather can fetch h[src] per edge.
  - Per 128-dst-node chunk: gather source rows, build one-hot S[e,d] via
    tensor_scalar(is_equal) against an iota row, broadcast ar[dst] to edges
    with a PE matmul (lhsT = S^T), compute p = max(exp(z), exp(0.2*z))
    (== exp(leaky_relu(z)) exactly, by monotonicity), scale messages
    in-place with a broadcast-AP multiply, and segment-sum via
    PSUM-accumulated matmuls out_chunk += S^T @ [p*h | p].
  - Epilogue per chunk: divide by the summed p (softmax denominator), apply
    bias + ELU, and immediately run the layer-2 node matmul for those rows.

Self-contained: only needs numpy/ml_dtypes plus the concourse runtime at
/opt/trn_rl_repo (the environment's Bass installation).
"""

import os
import sys

sys.path.insert(0, "/opt/trn_rl_repo")

import numpy as np
import ml_dtypes

from concourse import bass, bacc, mybir, tile
from concourse import bass_utils

BF16 = ml_dtypes.bfloat16

# ----------------------------------------------------------------------------
# configuration
# ----------------------------------------------------------------------------

def full_cfg():
    return dict(
        N=100000, F_IN=128, HID=16, HEADS=8, NCLS=40, NEG=0.2,
        NCORES=8, CHUNKS=98, BUCKET_ROWS=32768, GATHER="q7", GCAP=100000, GSP=False,
        NQ=4, AGSL=int(os.environ.get("GAT_AGSL", "0") or 0),
        DDS=32768,
    )


def pd_cfg():
    return dict(
        N=100000, F_IN=128, HID=16, HEADS=8, NCLS=40, NEG=0.2,
        NCORES=8, CHUNKS=98, BUCKET_ROWS=25088, NQ=4,
        SCHEME="pd64", DCH=64, ARLO=True,
    )


def derive(cfg):
    c = dict(cfg)
    c["SHARD"] = c["CHUNKS"] * 128
    c["NPAD"] = c["NCORES"] * c["SHARD"]
    c["NBUCK"] = -(-c["NPAD"] // c["BUCKET_ROWS"])
    if c.get("SCHEME") == "pd64":
        c["ECH"] = c["SHARD"] // c["DCH"]
    # L1 table row: [h (128) | pslot (8) | al_f32 (16) | ar_f32 (16) | pad] bf16
    c["ROW1"] = 256
    c["H1"] = c["HEADS"] * c["HID"]          # 128
    c["MSG1"] = c["H1"] + c["HEADS"]          # 136
    c["ALF1"] = (c["H1"] + 8) // 2            # f32 col 68
    c["ARF1"] = c["ALF1"] + 8                 # f32 col 76
    # L2 table row: [h2 (320) | pslot (8) | al_f32 (16) | ar_f32 (16) | pad] bf16
    c["ROW2"] = 384
    c["H2"] = c["HEADS"] * c["NCLS"]          # 320
    c["MSG2"] = c["H2"] + c["HEADS"]          # 328
    c["ALF2"] = (c["H2"] + 8) // 2            # f32 col 164
    c["ARF2"] = c["ALF2"] + 8                 # f32 col 172
    return c


# ----------------------------------------------------------------------------
# host-side weight preparation
# ----------------------------------------------------------------------------

def prep_weights(cfg, x, W1, a_src1, a_dst1, b1, W2, a_src2, a_dst2, b2):
    c = cfg
    H, HID, NCLS = c["HEADS"], c["HID"], c["NCLS"]
    F = c["F_IN"]

    x = np.asarray(x, np.float32)
    W1 = np.asarray(W1, np.float32)
    W2 = np.asarray(W2, np.float32)
    a_src1 = np.asarray(a_src1, np.float32); a_dst1 = np.asarray(a_dst1, np.float32)
    a_src2 = np.asarray(a_src2, np.float32); a_dst2 = np.asarray(a_dst2, np.float32)
    b1 = np.asarray(b1, np.float32); b2 = np.asarray(b2, np.float32)

    # layer1: channel-major column permutation: col c*H+h <- original h*HID+c
    cm1 = np.array([h * HID + cc for cc in range(HID) for h in range(H)])
    W1cm = W1[:, cm1]                                   # [F, 128]
    Wal1 = np.einsum("fhc,hc->fh", W1.reshape(F, H, HID), a_src1)
    War1 = np.einsum("fhc,hc->fh", W1.reshape(F, H, HID), a_dst1)
    W1cat = np.concatenate([W1cm, Wal1, War1], axis=1)  # [F, 144]

    # layer2 rows must be permuted to match z1's channel-major feature order
    cm2 = np.array([h * NCLS + cc for cc in range(NCLS) for h in range(H)])
    W2cm = W2[:, cm2]                                   # [128, 320]
    Wal2 = np.einsum("fhc,hc->fh", W2.reshape(c["H1"], H, NCLS), a_src2)
    War2 = np.einsum("fhc,hc->fh", W2.reshape(c["H1"], H, NCLS), a_dst2)
    W2cat = np.concatenate([W2cm, Wal2, War2], axis=1)[cm1, :]  # [128, 336]

    b1cm = b1[cm1]

    xT = np.zeros((F, c["NPAD"]), np.float32)
    xT[:, : x.shape[0]] = x.T
    return dict(
        xT=xT.astype(BF16),
        W1cat=W1cat.astype(BF16),
        W2cat=W2cat.astype(BF16),
        b1rep=np.tile(b1cm, (128, 1)).astype(np.float32),
        b2rep=np.tile(b2, (128, 1)).astype(np.float32),
        iota=np.tile(np.arange(128, dtype=np.float32), (128, 1)).astype(BF16),
        ident=np.eye(128, dtype=np.float32).astype(BF16),
        iota_rep=None,  # filled in run() once NBmax is known
    )


# ----------------------------------------------------------------------------
# host-side edge preparation
# ----------------------------------------------------------------------------

class EdgeMeta:
    pass


def prep_edges_indirect(cfg, src, dst):
    """No-bucket variant: int32 row offsets for indirect_dma_start.
    Streams: offs [128, BCOLS] int32 (src row ids), drel [128, BCOLS] f32."""
    c = cfg
    NC, CH, SHARD = c["NCORES"], c["CHUNKS"], c["SHARD"]
    src = np.asarray(src, np.int64)
    dst = np.asarray(dst, np.int64)
    core = dst // SHARD
    chunk = (dst % SHARD) // 128
    rel = (dst % 128).astype(np.float32)
    key = (core * CH + chunk).astype(np.int64)
    order = np.argsort(key, kind="stable")
    s_s = src[order]
    rel_s = rel[order]
    counts = np.bincount(key[order], minlength=NC * CH).reshape(NC, CH)
    offs = np.zeros(NC * CH + 1, np.int64)
    np.cumsum(counts.reshape(-1), out=offs[1:])

    L = counts.max(axis=0)
    L = np.maximum(((L + 127) // 128) * 128, 128)
    NB_k = (L // 128).astype(int)
    assert NB_k.max() * c["HEADS"] <= 512, f"zp psum overflow: NB={NB_k.max()}"

    meta = EdgeMeta()
    meta.NBmax = int(NB_k.max())
    meta.chunks = []
    bcol = 0
    for k in range(CH):
        meta.chunks.append(dict(NB=int(NB_k[k]), bcol=bcol))
        bcol += int(NB_k[k])
    meta.BCOLS = bcol

    streams = []
    for cc in range(NC):
        off_cols = []
        drel_cols = []
        for k in range(CH):
            Lk = int(L[k])
            o0 = offs[cc * CH + k]
            o1 = offs[cc * CH + k + 1]
            n = int(o1 - o0)
            ids = np.zeros(Lk, np.int32)
            relv = np.full(Lk, -1.0, np.float32)
            if n > 0:
                ids[:n] = s_s[o0:o1]
                relv[:n] = rel_s[o0:o1]
            off_cols.append(ids.reshape(-1, 128).T)
            drel_cols.append(relv.reshape(-1, 128).T.astype(BF16))
        streams.append(dict(
            offs=np.ascontiguousarray(np.concatenate(off_cols, axis=1)),
            drel=np.ascontiguousarray(np.concatenate(drel_cols, axis=1)),
        ))
    assert streams[0]["offs"].shape[1] == meta.BCOLS
    return meta, streams


def prep_edges(cfg, src, dst):
    """Sort/bucket edges; build per-core idx + dst_rel streams with padding
    that is uniform across cores (the SPMD program is shared)."""
    if cfg.get("GATHER") == "indirect":
        return prep_edges_indirect(cfg, src, dst)
    c = cfg
    NC, CH, BR, NBUCK, SHARD = c["NCORES"], c["CHUNKS"], c["BUCKET_ROWS"], c["NBUCK"], c["SHARD"]

    src = np.asarray(src, np.int64)
    dst = np.asarray(dst, np.int64)

    core = dst // SHARD
    chunk = (dst % SHARD) // 128
    rel = (dst % 128).astype(np.float32)
    buck = src // BR
    key = ((core * CH + chunk) * NBUCK + buck).astype(np.int64)
    order = np.argsort(key, kind="stable")
    s_s = src[order]
    rel_s = rel[order]
    key_s = key[order]

    counts = np.bincount(key_s, minlength=NC * CH * NBUCK).reshape(NC, CH, NBUCK)
    offs = np.zeros(NC * CH * NBUCK + 1, np.int64)
    np.cumsum(counts.reshape(-1), out=offs[1:])

    # uniform (max-over-cores) padded sizes per (chunk, bucket)
    L = counts.max(axis=0)                     # [CH, NBUCK]
    L = ((L + 127) // 128) * 128
    empty = L.sum(axis=1) == 0
    L[empty, 0] = 128

    NB_k = (L.sum(axis=1) // 128).astype(int)  # batches per chunk
    assert NB_k.max() * c["HEADS"] <= 512, f"zp psum overflow: NB={NB_k.max()}"

    meta = EdgeMeta()
    meta.NBmax = int(NB_k.max())
    meta.chunks = []
    icol = 0
    bcol = 0
    for k in range(CH):
        buckets = []
        io_local = 0
        for b in range(NBUCK):
            Lkb = int(L[k, b])
            if Lkb == 0:
                continue
            buckets.append(dict(b=b, L=Lkb, icol=icol, io_local=io_local))
            icol += Lkb // 16
            io_local += Lkb // 16
        meta.chunks.append(dict(NB=int(NB_k[k]), bcol=bcol, buckets=buckets,
                                icol0=buckets[0]["icol"], icols=io_local))
        bcol += int(NB_k[k])
    meta.ICOLS = icol
    meta.BCOLS = bcol
    meta.ckmax = max(ch["icols"] for ch in meta.chunks)

    # build per-core streams
    streams = []
    for cc in range(NC):
        idx_blocks = []
        drel_cols = []
        for k in range(CH):
            chunk_rel = []
            for binfo in meta.chunks[k]["buckets"]:
                b = binfo["b"]; Lkb = binfo["L"]
                o0 = offs[(cc * CH + k) * NBUCK + b]
                o1 = offs[(cc * CH + k) * NBUCK + b + 1]
                n = int(o1 - o0)
                idx16 = np.zeros(Lkb, np.int16)
                relv = np.full(Lkb, -1.0, np.float32)
                if n > 0:
                    idx16[:n] = (s_s[o0:o1] - b * BR).astype(np.int16)
                    relv[:n] = rel_s[o0:o1]
                wrapped = idx16.reshape(-1, 16).T          # [16, L/16]
                idx_blocks.append(np.tile(wrapped, (8, 1)))  # [128, L/16]
                chunk_rel.append(relv)
            allrel = np.concatenate(chunk_rel)
            drel_cols.append(allrel.reshape(-1, 128).T.astype(BF16))  # [128, NB_k]
        streams.append(dict(
            idx=np.concatenate(idx_blocks, axis=1),
            drel=np.concatenate(drel_cols, axis=1),
        ))
    assert streams[0]["idx"].shape[1] == meta.ICOLS
    assert streams[0]["drel"].shape[1] == meta.BCOLS
    return meta, streams


# ----------------------------------------------------------------------------
# kernel builder
# ----------------------------------------------------------------------------

def _ap(base, dims, extra_offset=0):
    return bass.AP(base.tensor, base.offset + extra_offset, dims)


def build_kernel(cfg, meta):
    c = cfg
    CH, NBUCK, BR = c["CHUNKS"], c["NBUCK"], c["BUCKET_ROWS"]
    SHARD, NPAD, NC = c["SHARD"], c["NPAD"], c["NCORES"]
    H = c["HEADS"]
    NEG = c["NEG"]
    bf = mybir.dt.bfloat16
    f32 = mybir.dt.float32
    EPS = 1e-30
    Copy = mybir.ActivationFunctionType.Copy
    Exp = mybir.ActivationFunctionType.Exp

    NQ = c.get("NQ", 1)
    nc = bacc.Bacc("TRN2", target_bir_lowering=False, debug=False, num_devices=NC,
                   num_swdge_queues=NQ,
                   dynamic_dma_scratch_size=c.get("DDS", 16384))
    qctr = [0]

    def next_q():
        q = qctr[0] % NQ
        qctr[0] += 1
        return q

    xT = nc.dram_tensor("xT", [128, SHARD], bf, kind="ExternalInput")
    W1c = nc.dram_tensor("W1c", [128, 144], bf, kind="ExternalInput")
    W2c = nc.dram_tensor("W2c", [128, c["H2"] + 16], bf, kind="ExternalInput")
    iota = nc.dram_tensor("iota", [128, 128], bf, kind="ExternalInput")
    ident = nc.dram_tensor("ident", [128, 128], bf, kind="ExternalInput")
    b1r = nc.dram_tensor("b1r", [128, 128], f32, kind="ExternalInput")
    b2r = nc.dram_tensor("b2r", [128, c["NCLS"]], f32, kind="ExternalInput")
    indirect = c.get("GATHER") == "indirect"
    if indirect:
        offsT = nc.dram_tensor("offs", [128, meta.BCOLS], mybir.dt.int32, kind="ExternalInput")
        drel = nc.dram_tensor("drel", [128, meta.BCOLS], bf, kind="ExternalInput")
    else:
        idxs = nc.dram_tensor("idxs", [128, meta.ICOLS], mybir.dt.int16, kind="ExternalInput")
        drel = nc.dram_tensor("drel", [128, meta.BCOLS], bf, kind="ExternalInput")
    iotar = nc.dram_tensor("iotar", [128, meta.NBmax * 128], bf, kind="ExternalInput")
    out = nc.dram_tensor("out", [SHARD, c["NCLS"]], f32, kind="ExternalOutput")

    NBmax = meta.NBmax

    with tile.TileContext(nc) as tc:
        with (
            tc.tile_pool(name="dram", bufs=1, space="DRAM") as dram,
            tc.tile_pool(name="const", bufs=1) as cp,
        ):
            AGSL = c.get("AGSL", 0)
            shr = {} if AGSL else {"addr_space": "Shared"}
            T1l = dram.tile([SHARD, c["ROW1"]], bf)
            T1 = dram.tile([NPAD, c["ROW1"]], bf, **shr)
            T2l = dram.tile([SHARD, c["ROW2"]], bf)
            T2 = dram.tile([NPAD, c["ROW2"]], bf, **shr)
            if AGSL:
                NSL = CH // AGSL
                sc1 = [dram.tile([NC * AGSL * 128, c["ROW1"]], bf, addr_space="Shared",
                                 name=f"sc1_{i}") for i in range(NSL)]
                sc2 = [dram.tile([NC * AGSL * 128, c["ROW2"]], bf, addr_space="Shared",
                                 name=f"sc2_{i}") for i in range(NSL)]

            iota_t = cp.tile([128, 128], bf, tag="iota")
            nc.sync.dma_start(out=iota_t[:], in_=iota[:])
            ident_t = cp.tile([128, 128], bf, tag="ident")
            nc.sync.dma_start(out=ident_t[:], in_=ident[:])
            b1_t = cp.tile([128, 128], f32, tag="b1")
            nc.sync.dma_start(out=b1_t[:], in_=b1r[:])
            b2_t = cp.tile([128, c["NCLS"]], f32, tag="b2")
            nc.sync.dma_start(out=b2_t[:], in_=b2r[:])
            W1_t = cp.tile([128, 144], bf, tag="W1")
            nc.sync.dma_start(out=W1_t[:], in_=W1c[:])
            W2_t = cp.tile([128, c["H2"] + 16], bf, tag="W2")
            nc.sync.dma_start(out=W2_t[:], in_=W2c[:])
            drel_t = cp.tile([128, meta.BCOLS], bf, tag="drel")
            nc.sync.dma_start(out=drel_t[:], in_=drel[:])
            iotar_t = cp.tile([128, meta.NBmax * 128], bf, tag="iotar")
            nc.sync.dma_start(out=iotar_t[:], in_=iotar[:])
            if indirect:
                offs_t = cp.tile([128, meta.BCOLS], mybir.dt.int32, tag="offs")
                nc.sync.dma_start(out=offs_t[:], in_=offsT[:])
            ar1_t = cp.tile([128, CH, 8], f32, tag="ar1")
            ar2_t = cp.tile([128, CH, 8], f32, tag="ar2")
            # bf16 hi/lo split of ar for bf16 zp matmuls (hi + lo == f32 value)
            ar1h_t = cp.tile([128, CH, 8], bf, tag="ar1h")
            ar1l_t = cp.tile([128, CH, 8], bf, tag="ar1l")
            ar2h_t = cp.tile([128, CH, 8], bf, tag="ar2h")
            ar2l_t = cp.tile([128, CH, 8], bf, tag="ar2l")

            # ---------------- phase A: L1 node matmuls ----------------
            with (
                tc.tile_pool(name="pa", bufs=2) as pa,
                tc.tile_pool(name="pap", bufs=2, space="PSUM") as pap,
            ):
                xT_t = pa.tile([128, SHARD], bf, tag="xT")
                nc.sync.dma_start(out=xT_t[:], in_=xT[:])
                SL = c.get("AGSL", 0)
                T1v = T1[:, :].rearrange("(cc s) r -> cc s r", cc=NC)
                for t in range(CH):
                    ps = pap.tile([128, 144], f32, tag="psA")
                    nc.tensor.matmul(ps[:], xT_t[:, t * 128:(t + 1) * 128], W1_t[:],
                                     start=True, stop=True)
                    row = pa.tile([128, c["MSG1"]], bf, tag="rowA")
                    nc.scalar.activation(out=row[:], in_=ps[:, 0:c["MSG1"]], func=Copy)
                    nc.sync.dma_start(out=T1l[t * 128:(t + 1) * 128, 0:c["MSG1"]], in_=row[:])
                    rowf = pa.tile([128, 16], f32, tag="rowAf")
                    nc.scalar.activation(out=rowf[:], in_=ps[:, c["H1"]:c["H1"] + 16], func=Copy)
                    t1l_f32 = T1l[t * 128:(t + 1) * 128, :].bitcast(f32)
                    nc.sync.dma_start(out=t1l_f32[:, c["ALF1"]:c["ALF1"] + 16], in_=rowf[:])
                    if SL and (t + 1) % SL == 0:
                        r0, r1 = (t + 1 - SL) * 128, (t + 1) * 128
                        sct = sc1[(t + 1) // SL - 1]
                        nc.gpsimd.collective_compute(
                            "AllGather", mybir.AluOpType.bypass,
                            replica_groups=[list(range(NC))],
                            ins=[T1l[r0:r1, :]], outs=[sct.opt()],
                        )
                        scv = sct[:, :].rearrange("(cc q) r -> cc q r", cc=NC)
                        nc.sync.dma_start(out=T1v[:, r0:r1, :], in_=scv)

            # ar1 preload (from local slice; rows (k*128+p), f32 cols ARF1..)
            t1l_all_f32 = T1l[:, :].bitcast(f32)
            ar1_src = t1l_all_f32.rearrange("(k p) r -> p k r", p=128)[:, :, c["ARF1"]:c["ARF1"] + 8]
            nc.sync.dma_start(out=ar1_t[:], in_=ar1_src)
            nc.scalar.activation(out=ar1h_t[:], in_=ar1_t[:], func=Copy)
            nc.vector.tensor_tensor(out=ar1l_t[:], in0=ar1_t[:], in1=ar1h_t[:],
                                    op=mybir.AluOpType.subtract)

            if not c.get("AGSL", 0):
                nc.gpsimd.collective_compute(
                    "AllGather", mybir.AluOpType.bypass,
                    replica_groups=[list(range(NC))],
                    ins=[T1l.opt()], outs=[T1.opt()],
                )

            def edge_chunk(k, lay, pool, psp):
                """Edge aggregation for chunk k of layer `lay`. Returns psum
                tile [128, MSG] holding [sum p*h | sum p] for the 128 dsts."""
                ROW = c["ROW1"] if lay == 1 else c["ROW2"]
                MSG = c["MSG1"] if lay == 1 else c["MSG2"]
                HW_ = c["H1"] if lay == 1 else c["H2"]
                ALF = c["ALF1"] if lay == 1 else c["ALF2"]
                CG = HW_ // 8
                tbl = T1 if lay == 1 else T2
                arh_t = ar1h_t if lay == 1 else ar2h_t
                arl_t = ar1l_t if lay == 1 else ar2l_t
                info = meta.chunks[k]
                NB = info["NB"]

                G = pool.tile([128, NBmax, ROW], bf, tag=f"G{lay}")
                if indirect:
                    nc.gpsimd.indirect_dma_start(
                        out=G[:, 0:NB, :],
                        out_offset=None,
                        in_=tbl[:, :],
                        in_offset=bass.IndirectOffsetOnAxis(
                            ap=offs_t[:, info["bcol"]:info["bcol"] + NB], axis=0),
                        queue_num=next_q(),
                    )
                else:
                    idx_t = pool.tile([128, meta.ckmax], mybir.dt.int16, tag=f"idx{lay}")
                    nc.sync.dma_start(out=idx_t[:, 0:info["icols"]],
                                      in_=idxs[:, info["icol0"]:info["icol0"] + info["icols"]])
                    gcap = c.get("GCAP", 1024)
                    slot = 0
                    for binfo in info["buckets"]:
                        b, L = binfo["b"], binfo["L"]
                        r0 = b * BR
                        r1 = min((b + 1) * BR, NPAD)
                        done = 0
                        while done < L:
                            Lp = min(gcap, L - done)
                            nb = Lp // 128
                            io = binfo["io_local"] + done // 16
                            nc.gpsimd.dma_gather(
                                out_ap=G[:, slot:slot + nb, :],
                                in_ap=tbl[r0:r1, :],
                                idxs_ap=idx_t[:, io:io + Lp // 16],
                                num_idxs=Lp, num_idxs_reg=Lp, elem_size=ROW,
                                single_packet=c.get("GSP", True),
                                queue_num=next_q(),
                            )
                            slot += nb
                            done += Lp

                Sall = pool.tile([128, NBmax, 128], bf, tag=f"S{lay}")
                zp = psp.tile([128, NBmax * 8], f32, tag="zp")
                db = drel_t[:, info["bcol"]:info["bcol"] + NB]
                nc.vector.tensor_tensor(
                    out=Sall[:, 0:NB, :],
                    in0=_ap(iotar_t[:, :], [iotar_t[:, :].ap[0], [128, NB], [1, 128]]),
                    in1=_ap(db, [db.ap[0], [1, NB], [0, 128]]),
                    op=mybir.AluOpType.is_equal,
                )
                ARLO = c.get("ARLO", True)
                GRP = c.get("GRP", 8)
                for j0 in range(0, NB, GRP):
                    g = min(GRP, NB - j0)
                    STg = psp.tile([128, GRP * 128], bf, tag="STg")
                    for j in range(j0, j0 + g):
                        nc.tensor.transpose(out=STg[:, (j - j0) * 128:(j - j0 + 1) * 128],
                                            in_=Sall[:, j, :], identity=ident_t[:])
                    STs = pool.tile([128, GRP * 128], bf, tag="STs")
                    nc.scalar.activation(out=STs[:, 0:g * 128], in_=STg[:, 0:g * 128], func=Copy)
                    for j in range(j0, j0 + g):
                        sl = STs[:, (j - j0) * 128:(j - j0 + 1) * 128]
                        nc.tensor.matmul(zp[:, j * 8:(j + 1) * 8], sl, arh_t[:, k, :],
                                         start=True, stop=not ARLO)
                        if ARLO:
                            nc.tensor.matmul(zp[:, j * 8:(j + 1) * 8], sl, arl_t[:, k, :],
                                             start=False, stop=True)

                # z = al[src] + ar[dst]  (both f32)
                Gf = G[:, :, :].bitcast(f32)   # [128, NBmax, ROW//2]
                al_view = Gf[:, 0:NB, ALF:ALF + 8]
                zs = pool.tile([128, NBmax, 8], f32, tag="zs")
                nc.vector.tensor_tensor(out=zs[:, 0:NB, :],
                                        in0=_ap(zp[:, 0:NB * 8], [zp[:, :].ap[0], [8, NB], [1, 8]]),
                                        in1=al_view, op=mybir.AluOpType.add)
                # p = max(exp(z), exp(0.2 z)) == exp(leaky_relu(z))
                e1 = pool.tile([128, NBmax, 8], f32, tag="e1")
                nc.scalar.activation(out=e1[:, 0:NB, :], in_=zs[:, 0:NB, :], func=Exp)
                e2 = pool.tile([128, NBmax, 8], f32, tag="e2")
                nc.scalar.activation(out=e2[:, 0:NB, :], in_=zs[:, 0:NB, :], func=Exp, scale=NEG)
                nc.vector.tensor_tensor(out=G[:, 0:NB, HW_:HW_ + 8], in0=e1[:, 0:NB, :],
                                        in1=e2[:, 0:NB, :], op=mybir.AluOpType.max)

                # messages in-place: G[:, :, :HW] *= p (broadcast over CG)
                gbase = G[:, :, :]
                part = gbase.ap[0]
                h_view = _ap(gbase, [part, [ROW, NB], [8, CG], [1, 8]])
                p_view = _ap(gbase, [part, [ROW, NB], [0, CG], [1, 8]], extra_offset=HW_)
                nc.vector.tensor_tensor(out=h_view, in0=h_view, in1=p_view,
                                        op=mybir.AluOpType.mult)

                pso = psp.tile([128, MSG], f32, tag="oc")
                for j in range(NB):
                    nc.tensor.matmul(pso[:], Sall[:, j, :], G[:, j, 0:MSG],
                                     start=(j == 0), stop=(j == NB - 1))
                return pso

            # ---------------- phase B + C: L1 edges + L2 node rows ----------------
            with (
                tc.tile_pool(name="pb", bufs=3) as pb,
                tc.tile_pool(name="pbp", bufs=2, space="PSUM") as pbp,
            ):
                for k in range(CH):
                    pso = edge_chunk(k, 1, pb, pbp)
                    # epilogue: out1 = pso[:, :128] / (denom+eps); z1 = elu(out1+b1)
                    rec = pb.tile([128, 8], f32, tag="rec")
                    nc.vector.tensor_scalar(out=rec[:], in0=pso[:, c["H1"]:c["H1"] + 8],
                                            scalar1=EPS, scalar2=None, op0=mybir.AluOpType.add)
                    nc.vector.reciprocal(out=rec[:], in_=rec[:])
                    t1v = pb.tile([128, 128], f32, tag="t1v")
                    rb = rec[:, :]
                    rec_b = _ap(rb, [rb.ap[0], [0, 16], [1, 8]])
                    nc.vector.tensor_tensor(
                        out=t1v[:, :], in0=pso[:, 0:c["H1"]],
                        in1=rec_b, op=mybir.AluOpType.mult)
                    nc.vector.tensor_tensor(out=t1v[:], in0=t1v[:], in1=b1_t[:],
                                            op=mybir.AluOpType.add)
                    # elu: z1 = max(t,0) + exp(min(t,0)) - 1
                    mn = pb.tile([128, 128], f32, tag="mn")
                    nc.vector.tensor_scalar(out=mn[:], in0=t1v[:], scalar1=0.0,
                                            scalar2=None, op0=mybir.AluOpType.min)
                    ex = pb.tile([128, 128], f32, tag="ex")
                    nc.scalar.activation(out=ex[:], in_=mn[:], func=Exp)
                    nc.vector.tensor_scalar(out=ex[:], in0=ex[:], scalar1=-1.0,
                                            scalar2=None, op0=mybir.AluOpType.add)
                    nc.vector.tensor_scalar(out=t1v[:], in0=t1v[:], scalar1=0.0,
                                            scalar2=None, op0=mybir.AluOpType.max)
                    z1 = pb.tile([128, 128], bf, tag="z1")
                    nc.vector.tensor_tensor(out=z1[:], in0=t1v[:], in1=ex[:],
                                            op=mybir.AluOpType.add)
                    # phase C: h2cat rows for these 128 nodes
                    z1Tp = pbp.tile([128, c.get("GRP", 8) * 128], bf, tag="STg")
                    nc.tensor.transpose(out=z1Tp[:, 0:128], in_=z1[:], identity=ident_t[:])
                    z1T = pb.tile([128, 128], bf, tag="z1T")
                    nc.scalar.activation(out=z1T[:], in_=z1Tp[:, 0:128], func=Copy)
                    h2p = pbp.tile([128, c["H2"] + 16], f32, tag="h2")
                    nc.tensor.matmul(h2p[:], z1T[:], W2_t[:], start=True, stop=True)
                    row2 = pb.tile([128, c["MSG2"]], bf, tag="row2")
                    nc.scalar.activation(out=row2[:], in_=h2p[:, 0:c["MSG2"]], func=Copy)
                    nc.sync.dma_start(out=T2l[k * 128:(k + 1) * 128, 0:c["MSG2"]], in_=row2[:])
                    row2f = pb.tile([128, 16], f32, tag="row2f")
                    nc.scalar.activation(out=row2f[:], in_=h2p[:, c["H2"]:c["H2"] + 16], func=Copy)
                    t2l_f32 = T2l[k * 128:(k + 1) * 128, :].bitcast(f32)
                    nc.sync.dma_start(out=t2l_f32[:, c["ALF2"]:c["ALF2"] + 16], in_=row2f[:])
                    SL = c.get("AGSL", 0)
                    if SL and (k + 1) % SL == 0:
                        r0, r1 = (k + 1 - SL) * 128, (k + 1) * 128
                        T2v = T2[:, :].rearrange("(cc s) r -> cc s r", cc=NC)
                        sck = sc2[(k + 1) // SL - 1]
                        nc.gpsimd.collective_compute(
                            "AllGather", mybir.AluOpType.bypass,
                            replica_groups=[list(range(NC))],
                            ins=[T2l[r0:r1, :]], outs=[sck.opt()],
                        )
                        scv2 = sck[:, :].rearrange("(cc q) r -> cc q r", cc=NC)
                        nc.sync.dma_start(out=T2v[:, r0:r1, :], in_=scv2)

            t2l_all_f32 = T2l[:, :].bitcast(f32)
            ar2_src = t2l_all_f32.rearrange("(k p) r -> p k r", p=128)[:, :, c["ARF2"]:c["ARF2"] + 8]
            nc.sync.dma_start(out=ar2_t[:], in_=ar2_src)
            nc.scalar.activation(out=ar2h_t[:], in_=ar2_t[:], func=Copy)
            nc.vector.tensor_tensor(out=ar2l_t[:], in0=ar2_t[:], in1=ar2h_t[:],
                                    op=mybir.AluOpType.subtract)

            if not c.get("AGSL", 0):
                nc.gpsimd.collective_compute(
                    "AllGather", mybir.AluOpType.bypass,
                    replica_groups=[list(range(NC))],
                    ins=[T2l.opt()], outs=[T2.opt()],
                )

            # ---------------- phase D: L2 edges + final output ----------------
            with (
                tc.tile_pool(name="pd", bufs=3) as pd,
                tc.tile_pool(name="pdp", bufs=2, space="PSUM") as pdp,
            ):
                for k in range(CH):
                    pso = edge_chunk(k, 2, pd, pdp)
                    rec = pd.tile([128, 8], f32, tag="rec2")
                    nc.vector.tensor_scalar(out=rec[:], in0=pso[:, c["H2"]:c["H2"] + 8],
                                            scalar1=EPS, scalar2=None, op0=mybir.AluOpType.add)
                    nc.vector.reciprocal(out=rec[:], in_=rec[:])
                    tv = pd.tile([128, c["H2"]], f32, tag="tv")
                    rb = rec[:, :]
                    rec_b = _ap(rb, [rb.ap[0], [0, c["NCLS"]], [1, 8]])
                    nc.vector.tensor_tensor(
                        out=tv[:, :], in0=pso[:, 0:c["H2"]],
                        in1=rec_b, op=mybir.AluOpType.mult)
                    red = pd.tile([128, c["NCLS"]], f32, tag="red")
                    nc.vector.tensor_reduce(
                        out=red[:],
                        in_=_ap(tv[:, :], [tv[:, :].ap[0], [8, c["NCLS"]], [1, 8]]),
                        axis=mybir.AxisListType.X, op=mybir.AluOpType.add)
                    nc.vector.tensor_scalar(out=red[:], in0=red[:], scalar1=1.0 / H,
                                            scalar2=None, op0=mybir.AluOpType.mult)
                    ot = pd.tile([128, c["NCLS"]], f32, tag="ot")
                    nc.vector.tensor_tensor(out=ot[:], in0=red[:], in1=b2_t[:],
                                            op=mybir.AluOpType.add)
                    nc.sync.dma_start(out=out[k * 128:(k + 1) * 128, :], in_=ot[:])

    nc.compile()
    return nc


# ----------------------------------------------------------------------------
# pd64 scheme: 64-dst chunks, one dst per partition per (chunk, bucket)
# ----------------------------------------------------------------------------

def _min_nb(cnts, nparts=128):
    n = max(1, int(np.ceil(cnts.sum() / nparts)))
    while np.ceil(cnts / n).sum() > nparts:
        n += 1
    return n


def prep_edges_pd(cfg, src, dst):
    c = cfg
    NC, CH, DCH = c["NCORES"], c["ECH"], c["DCH"]
    BR, NBUCK, SHARD = c["BUCKET_ROWS"], c["NBUCK"], c["SHARD"]

    src = np.asarray(src, np.int64)
    dst = np.asarray(dst, np.int64)
    core = dst // SHARD
    chunk = (dst % SHARD) // DCH
    rel = dst % DCH
    buck = src // BR

    key = (((core * CH + chunk) * NBUCK + buck) * DCH + rel)
    order = np.lexsort((src, key))
    s_s = src[order]
    key_s = key[order]
    counts = np.bincount(key_s, minlength=NC * CH * NBUCK * DCH).reshape(
        NC, CH, NBUCK, DCH)
    offs = np.zeros(NC * CH * NBUCK * DCH + 1, np.int64)
    np.cumsum(counts.reshape(-1), out=offs[1:])

    NBb = np.zeros((CH, NBUCK), int)
    for k in range(CH):
        for b in range(NBUCK):
            NBb[k, b] = max(_min_nb(counts[cc, k, b]) for cc in range(NC))

    meta = EdgeMeta()
    meta.chunks = []
    icol = 0
    bcol = 0
    for k in range(CH):
        segs = []
        slot = 0
        for b in range(NBUCK):
            nb = int(NBb[k, b])
            segs.append(dict(b=b, nb=nb, slot=slot, icol=icol))
            slot += nb
            icol += nb * 8
        meta.chunks.append(dict(NB=slot, bcol=bcol, segs=segs))
        bcol += slot
    meta.ICOLS = icol
    meta.BCOLS = bcol
    meta.NBmax = max(ch["NB"] for ch in meta.chunks)
    meta.ckicols = max(sum(s["nb"] * 8 for s in ch["segs"]) for ch in meta.chunks)

    streams = []
    for cc in range(NC):
        idx_cols = []
        mask_cols = []
        pdst_cols = []
        for k in range(CH):
            for seg in meta.chunks[k]["segs"]:
                b, nb = seg["b"], seg["nb"]
                cnt = counts[cc, k, b]
                part_rel = np.full(128, -1, np.int64)
                grid_src = np.zeros((128, nb), np.int64)
                valid = np.zeros((128, nb), bool)
                p = 0
                for r in range(DCH):
                    n = int(cnt[r])
                    if n == 0:
                        continue
                    o0 = offs[((cc * CH + k) * NBUCK + b) * DCH + r]
                    edges = s_s[o0:o0 + n] - b * BR
                    pos = 0
                    while pos < n:
                        take = min(nb, n - pos)
                        assert p < 128, f"pd pack overflow c{cc} k{k} b{b}"
                        part_rel[p] = r
                        grid_src[p, :take] = edges[pos:pos + take]
                        valid[p, :take] = True
                        pos += take
                        p += 1
                idx16 = grid_src.T.reshape(-1).astype(np.int16)
                wrapped = idx16.reshape(-1, 16).T
                idx_cols.append(np.tile(wrapped, (8, 1)))
                mask_cols.append(valid.astype(np.float32).astype(BF16))
                pdst_cols.append(part_rel.astype(np.float32).astype(BF16))
        streams.append(dict(
            idx=np.ascontiguousarray(np.concatenate(idx_cols, axis=1)),
            mask=np.ascontiguousarray(np.concatenate(mask_cols, axis=1)),
            pdst=np.ascontiguousarray(np.stack(pdst_cols, axis=1)),
        ))
    assert streams[0]["idx"].shape[1] == meta.ICOLS
    assert streams[0]["mask"].shape[1] == meta.BCOLS
    return meta, streams


def build_kernel_pd(cfg, meta):
    c = cfg
    PCH, ECH, NBUCK, BR = c["CHUNKS"], c["ECH"], c["NBUCK"], c["BUCKET_ROWS"]
    SHARD, NPAD, NC, DCH = c["SHARD"], c["NPAD"], c["NCORES"], c["DCH"]
    H = c["HEADS"]
    NEG = c["NEG"]
    ARLO = c.get("ARLO", True)
    bf = mybir.dt.bfloat16
    f32 = mybir.dt.float32
    EPS = 1e-30
    Copy = mybir.ActivationFunctionType.Copy
    Exp = mybir.ActivationFunctionType.Exp
    NBmax = meta.NBmax

    NQ = c.get("NQ", 1)
    nc = bacc.Bacc("TRN2", target_bir_lowering=False, debug=False, num_devices=NC,
                   num_swdge_queues=NQ)
    qctr = [0]

    def next_q():
        q = qctr[0] % NQ
        qctr[0] += 1
        return q

    xT = nc.dram_tensor("xT", [128, SHARD], bf, kind="ExternalInput")
    W1c = nc.dram_tensor("W1c", [128, 144], bf, kind="ExternalInput")
    W2c = nc.dram_tensor("W2c", [128, c["H2"] + 16], bf, kind="ExternalInput")
    iota = nc.dram_tensor("iota", [128, 128], bf, kind="ExternalInput")
    ident = nc.dram_tensor("ident", [128, 128], bf, kind="ExternalInput")
    b1r = nc.dram_tensor("b1r", [128, 128], f32, kind="ExternalInput")
    b2r = nc.dram_tensor("b2r", [128, c["NCLS"]], f32, kind="ExternalInput")
    idxs = nc.dram_tensor("idxs", [128, meta.ICOLS], mybir.dt.int16, kind="ExternalInput")
    maskS = nc.dram_tensor("maskS", [128, meta.BCOLS], bf, kind="ExternalInput")
    pdstS = nc.dram_tensor("pdstS", [128, ECH * NBUCK], bf, kind="ExternalInput")
    out = nc.dram_tensor("out", [SHARD, c["NCLS"]], f32, kind="ExternalOutput")

    with tile.TileContext(nc) as tc:
        with (
            tc.tile_pool(name="dram", bufs=1, space="DRAM") as dram,
            tc.tile_pool(name="const", bufs=1) as cp,
        ):
            T1l = dram.tile([SHARD, c["ROW1"]], bf)
            T1 = dram.tile([NPAD, c["ROW1"]], bf, addr_space="Shared")
            T2l = dram.tile([SHARD, c["ROW2"]], bf)
            T2 = dram.tile([NPAD, c["ROW2"]], bf, addr_space="Shared")

            iota_t = cp.tile([128, 128], bf, tag="iota")
            nc.sync.dma_start(out=iota_t[:], in_=iota[:])
            ident_t = cp.tile([128, 128], bf, tag="ident")
            nc.sync.dma_start(out=ident_t[:], in_=ident[:])
            b1_t = cp.tile([128, 128], f32, tag="b1")
            nc.sync.dma_start(out=b1_t[:], in_=b1r[:])
            b2_t = cp.tile([128, c["NCLS"]], f32, tag="b2")
            nc.sync.dma_start(out=b2_t[:], in_=b2r[:])
            W1_t = cp.tile([128, 144], bf, tag="W1")
            nc.sync.dma_start(out=W1_t[:], in_=W1c[:])
            W2_t = cp.tile([128, c["H2"] + 16], bf, tag="W2")
            nc.sync.dma_start(out=W2_t[:], in_=W2c[:])
            mask_t = cp.tile([128, meta.BCOLS], bf, tag="mask")
            nc.sync.dma_start(out=mask_t[:], in_=maskS[:])
            pdst_t = cp.tile([128, ECH * NBUCK], bf, tag="pdst")
            nc.sync.dma_start(out=pdst_t[:], in_=pdstS[:])
            ar1_t = cp.tile([64, ECH, 8], f32, tag="ar1")
            ar2_t = cp.tile([64, ECH, 8], f32, tag="ar2")
            ar1h_t = cp.tile([64, ECH, 8], bf, tag="ar1h")
            ar1l_t = cp.tile([64, ECH, 8], bf, tag="ar1l")
            ar2h_t = cp.tile([64, ECH, 8], bf, tag="ar2h")
            ar2l_t = cp.tile([64, ECH, 8], bf, tag="ar2l")

            # ---------------- phase A: L1 node matmuls ----------------
            with (
                tc.tile_pool(name="pa", bufs=2) as pa,
                tc.tile_pool(name="pap", bufs=2, space="PSUM") as pap,
            ):
                xT_t = pa.tile([128, SHARD], bf, tag="xT")
                nc.sync.dma_start(out=xT_t[:], in_=xT[:])
                for t in range(PCH):
                    ps = pap.tile([128, 144], f32, tag="psA")
                    nc.tensor.matmul(ps[:], xT_t[:, t * 128:(t + 1) * 128], W1_t[:],
                                     start=True, stop=True)
                    row = pa.tile([128, c["MSG1"]], bf, tag="rowA")
                    nc.scalar.activation(out=row[:], in_=ps[:, 0:c["MSG1"]], func=Copy)
                    nc.sync.dma_start(out=T1l[t * 128:(t + 1) * 128, 0:c["MSG1"]], in_=row[:])
                    rowf = pa.tile([128, 16], f32, tag="rowAf")
                    nc.scalar.activation(out=rowf[:], in_=ps[:, c["H1"]:c["H1"] + 16], func=Copy)
                    t1l_f32 = T1l[t * 128:(t + 1) * 128, :].bitcast(f32)
                    nc.sync.dma_start(out=t1l_f32[:, c["ALF1"]:c["ALF1"] + 16], in_=rowf[:])

            t1l_all_f32 = T1l[:, :].bitcast(f32)
            ar1_src = t1l_all_f32.rearrange("(k p) r -> p k r", p=DCH)[:, :, c["ARF1"]:c["ARF1"] + 8]
            nc.sync.dma_start(out=ar1_t[:], in_=ar1_src)
            nc.scalar.activation(out=ar1h_t[:], in_=ar1_t[:], func=Copy)
            nc.vector.tensor_tensor(out=ar1l_t[:], in0=ar1_t[:], in1=ar1h_t[:],
                                    op=mybir.AluOpType.subtract)

            nc.gpsimd.collective_compute(
                "AllGather", mybir.AluOpType.bypass,
                replica_groups=[list(range(NC))],
                ins=[T1l.opt()], outs=[T1.opt()],
            )

            def edge_chunk(k, lay, pool, psp, psp1):
                ROW = c["ROW1"] if lay == 1 else c["ROW2"]
                MSG = c["MSG1"] if lay == 1 else c["MSG2"]
                HW_ = c["H1"] if lay == 1 else c["H2"]
                ALF = c["ALF1"] if lay == 1 else c["ALF2"]
                CG = HW_ // 8
                tbl = T1 if lay == 1 else T2
                arh_t = ar1h_t if lay == 1 else ar2h_t
                arl_t = ar1l_t if lay == 1 else ar2l_t
                info = meta.chunks[k]
                NB = info["NB"]
                segs = info["segs"]
                icol0 = segs[0]["icol"]
                icols = sum(s["nb"] * 8 for s in segs)

                G = pool.tile([128, NBmax, ROW], bf, tag=f"G{lay}")
                idx_t = pool.tile([128, meta.ckicols], mybir.dt.int16, tag=f"idx{lay}")
                nc.sync.dma_start(out=idx_t[:, 0:icols],
                                  in_=idxs[:, icol0:icol0 + icols])
                for seg in segs:
                    b, nb, slot = seg["b"], seg["nb"], seg["slot"]
                    io = seg["icol"] - icol0
                    nc.gpsimd.dma_gather(
                        out_ap=G[:, slot:slot + nb, :],
                        in_ap=tbl[b * BR:(b + 1) * BR, :],
                        idxs_ap=idx_t[:, io:io + nb * 8],
                        num_idxs=nb * 128, num_idxs_reg=nb * 128, elem_size=ROW,
                        single_packet=False,
                        queue_num=next_q(),
                    )

                Gf = G[:, :, :].bitcast(f32)
                zs = pool.tile([128, NBmax, 8], f32, tag="zs")
                pms = []
                for si, seg in enumerate(segs):
                    b, nb, slot = seg["b"], seg["nb"], seg["slot"]
                    Pm = pool.tile([128, 64], bf, tag=f"Pm{si}")
                    pd = pdst_t[:, k * NBUCK + b:k * NBUCK + b + 1]
                    nc.vector.tensor_tensor(
                        out=Pm[:], in0=iota_t[:, 0:64],
                        in1=_ap(pd, [pd.ap[0], [0, 64]]),
                        op=mybir.AluOpType.is_equal)
                    PmTp = psp1.tile([64, 128], bf, tag="PmTp")
                    nc.tensor.transpose(out=PmTp[:], in_=Pm[:], identity=ident_t[:])
                    PmT = pool.tile([64, 128], bf, tag="PmT")
                    nc.scalar.activation(out=PmT[:], in_=PmTp[:], func=Copy)
                    arp_ps = psp1.tile([128, 8], f32, tag="arp")
                    nc.tensor.matmul(arp_ps[:], PmT[:], arh_t[0:64, k, :],
                                     start=True, stop=not ARLO)
                    if ARLO:
                        nc.tensor.matmul(arp_ps[:], PmT[:], arl_t[0:64, k, :],
                                         start=False, stop=True)
                    arp = pool.tile([128, 8], f32, tag=f"arp{si}")
                    nc.scalar.activation(out=arp[:], in_=arp_ps[:], func=Copy)
                    a_v = arp[:, :]
                    nc.vector.tensor_tensor(
                        out=zs[:, slot:slot + nb, :],
                        in0=Gf[:, slot:slot + nb, ALF:ALF + 8],
                        in1=_ap(a_v, [a_v.ap[0], [0, nb], [1, 8]]),
                        op=mybir.AluOpType.add)
                    pms.append(Pm)

                e1 = pool.tile([128, NBmax, 8], f32, tag="e1")
                nc.scalar.activation(out=e1[:, 0:NB, :], in_=zs[:, 0:NB, :], func=Exp)
                e2 = pool.tile([128, NBmax, 8], f32, tag="e2")
                nc.scalar.activation(out=e2[:, 0:NB, :], in_=zs[:, 0:NB, :], func=Exp, scale=NEG)
                pmx = pool.tile([128, NBmax, 8], f32, tag="pmx")
                nc.vector.tensor_tensor(out=pmx[:, 0:NB, :], in0=e1[:, 0:NB, :],
                                        in1=e2[:, 0:NB, :], op=mybir.AluOpType.max)
                msk = mask_t[:, info["bcol"]:info["bcol"] + NB]
                nc.vector.tensor_tensor(
                    out=G[:, 0:NB, HW_:HW_ + 8], in0=pmx[:, 0:NB, :],
                    in1=_ap(msk, [msk.ap[0], [1, NB], [0, 8]]),
                    op=mybir.AluOpType.mult)

                gbase = G[:, :, :]
                part = gbase.ap[0]
                h_view = _ap(gbase, [part, [ROW, NB], [8, CG], [1, 8]])
                p_view = _ap(gbase, [part, [ROW, NB], [0, CG], [1, 8]], extra_offset=HW_)
                nc.vector.tensor_tensor(out=h_view, in0=h_view, in1=p_view,
                                        op=mybir.AluOpType.mult)

                pso = psp.tile([64, MSG], f32, tag="oc")
                jg = 0
                for si, seg in enumerate(segs):
                    for j in range(seg["nb"]):
                        nc.tensor.matmul(pso[:], pms[si][:], G[:, seg["slot"] + j, 0:MSG],
                                         start=(jg == 0), stop=(jg == NB - 1))
                        jg += 1
                return pso

            # ---------------- phase B + C: L1 edges + L2 node rows ----------------
            with (
                tc.tile_pool(name="pb", bufs=2) as pb,
                tc.tile_pool(name="pbp", bufs=2, space="PSUM") as pbp,
                tc.tile_pool(name="pbp1", bufs=1, space="PSUM") as pbp1,
            ):
                for k in range(ECH):
                    pso = edge_chunk(k, 1, pb, pbp, pbp1)
                    rec = pb.tile([64, 8], f32, tag="rec")
                    nc.vector.tensor_scalar(out=rec[:], in0=pso[:, c["H1"]:c["H1"] + 8],
                                            scalar1=EPS, scalar2=None, op0=mybir.AluOpType.add)
                    nc.vector.reciprocal(out=rec[:], in_=rec[:])
                    t1v = pb.tile([64, 128], f32, tag="t1v")
                    rb = rec[:, :]
                    rec_b = _ap(rb, [rb.ap[0], [0, 16], [1, 8]])
                    nc.vector.tensor_tensor(
                        out=t1v[:, :], in0=pso[:, 0:c["H1"]],
                        in1=rec_b, op=mybir.AluOpType.mult)
                    nc.vector.tensor_tensor(out=t1v[:], in0=t1v[:], in1=b1_t[0:64, :],
                                            op=mybir.AluOpType.add)
                    mn = pb.tile([64, 128], f32, tag="mn")
                    nc.vector.tensor_scalar(out=mn[:], in0=t1v[:], scalar1=0.0,
                                            scalar2=None, op0=mybir.AluOpType.min)
                    ex = pb.tile([64, 128], f32, tag="ex")
                    nc.scalar.activation(out=ex[:], in_=mn[:], func=Exp)
                    nc.vector.tensor_scalar(out=ex[:], in0=ex[:], scalar1=-1.0,
                                            scalar2=None, op0=mybir.AluOpType.add)
                    nc.vector.tensor_scalar(out=t1v[:], in0=t1v[:], scalar1=0.0,
                                            scalar2=None, op0=mybir.AluOpType.max)
                    z1 = pb.tile([64, 128], bf, tag="z1")
                    nc.vector.tensor_tensor(out=z1[:], in0=t1v[:], in1=ex[:],
                                            op=mybir.AluOpType.add)
                    z1Tp = pbp1.tile([128, 64], bf, tag="zT")
                    nc.tensor.transpose(out=z1Tp[:], in_=z1[:], identity=ident_t[0:64, 0:64])
                    z1T = pb.tile([128, 64], bf, tag="z1T")
                    nc.scalar.activation(out=z1T[:], in_=z1Tp[:], func=Copy)
                    h2p = pbp1.tile([64, c["H2"] + 16], f32, tag="h2")
                    nc.tensor.matmul(h2p[:], z1T[:], W2_t[:], start=True, stop=True)
                    row2 = pb.tile([64, c["MSG2"]], bf, tag="row2")
                    nc.scalar.activation(out=row2[:], in_=h2p[:, 0:c["MSG2"]], func=Copy)
                    nc.sync.dma_start(out=T2l[k * DCH:(k + 1) * DCH, 0:c["MSG2"]], in_=row2[:])
                    row2f = pb.tile([64, 16], f32, tag="row2f")
                    nc.scalar.activation(out=row2f[:], in_=h2p[:, c["H2"]:c["H2"] + 16], func=Copy)
                    t2l_f32 = T2l[k * DCH:(k + 1) * DCH, :].bitcast(f32)
                    nc.sync.dma_start(out=t2l_f32[:, c["ALF2"]:c["ALF2"] + 16], in_=row2f[:])

            t2l_all_f32 = T2l[:, :].bitcast(f32)
            ar2_src = t2l_all_f32.rearrange("(k p) r -> p k r", p=DCH)[:, :, c["ARF2"]:c["ARF2"] + 8]
            nc.sync.dma_start(out=ar2_t[:], in_=ar2_src)
            nc.scalar.activation(out=ar2h_t[:], in_=ar2_t[:], func=Copy)
            nc.vector.tensor_tensor(out=ar2l_t[:], in0=ar2_t[:], in1=ar2h_t[:],
                                    op=mybir.AluOpType.subtract)

            nc.gpsimd.collective_compute(
                "AllGather", mybir.AluOpType.bypass,
                replica_groups=[list(range(NC))],
                ins=[T2l.opt()], outs=[T2.opt()],
            )

            # ---------------- phase D: L2 edges + final output ----------------
            with (
                tc.tile_pool(name="pd", bufs=2) as pd_,
                tc.tile_pool(name="pdp", bufs=2, space="PSUM") as pdp,
                tc.tile_pool(name="pdp1", bufs=1, space="PSUM") as pdp1,
            ):
                for k in range(ECH):
                    pso = edge_chunk(k, 2, pd_, pdp, pdp1)
                    rec = pd_.tile([64, 8], f32, tag="rec2")
                    nc.vector.tensor_scalar(out=rec[:], in0=pso[:, c["H2"]:c["H2"] + 8],
                                            scalar1=EPS, scalar2=None, op0=mybir.AluOpType.add)
                    nc.vector.reciprocal(out=rec[:], in_=rec[:])
                    tv = pd_.tile([64, c["H2"]], f32, tag="tv")
                    rb = rec[:, :]
                    rec_b = _ap(rb, [rb.ap[0], [0, c["NCLS"]], [1, 8]])
                    nc.vector.tensor_tensor(
                        out=tv[:, :], in0=pso[:, 0:c["H2"]],
                        in1=rec_b, op=mybir.AluOpType.mult)
                    red = pd_.tile([64, c["NCLS"]], f32, tag="red")
                    nc.vector.tensor_reduce(
                        out=red[:],
                        in_=_ap(tv[:, :], [tv[:, :].ap[0], [8, c["NCLS"]], [1, 8]]),
                        axis=mybir.AxisListType.X, op=mybir.AluOpType.add)
                    nc.vector.tensor_scalar(out=red[:], in0=red[:], scalar1=1.0 / H,
                                            scalar2=None, op0=mybir.AluOpType.mult)
                    ot = pd_.tile([64, c["NCLS"]], f32, tag="ot")
                    nc.vector.tensor_tensor(out=ot[:], in0=red[:], in1=b2_t[0:64, :],
                                            op=mybir.AluOpType.add)
                    nc.sync.dma_start(out=out[k * DCH:(k + 1) * DCH, :], in_=ot[:])

    nc.compile()
    return nc


def make_in_maps_pd(cfg, weights, meta, streams):
    c = cfg
    in_maps = []
    for cc in range(c["NCORES"]):
        in_maps.append(dict(
            xT=np.ascontiguousarray(weights["xT"][:, cc * c["SHARD"]:(cc + 1) * c["SHARD"]]),
            W1c=weights["W1cat"],
            W2c=weights["W2cat"],
            iota=weights["iota"],
            ident=weights["ident"],
            b1r=weights["b1rep"],
            b2r=weights["b2rep"],
            idxs=streams[cc]["idx"],
            maskS=streams[cc]["mask"],
            pdstS=streams[cc]["pdst"],
        ))
    return in_maps


# ----------------------------------------------------------------------------
# driver
# ----------------------------------------------------------------------------

def _install_ntff_shim():
    """Register the axon NTFF profiling hook (for trace=True runs)."""
    import types
    if "antenv.axon_hooks" not in sys.modules:
        try:
            import antenv  # noqa: F401
        except ImportError:
            pkg = types.ModuleType("antenv")
            pkg.__path__ = []
            sys.modules["antenv"] = pkg
        mod = types.ModuleType("antenv.axon_hooks")
        _hook = [None]
        mod.set_axon_ntff_profile_hook = lambda h: _hook.__setitem__(0, h)
        mod.get_axon_ntff_profile_hook = lambda: _hook[0]
        sys.modules["antenv"].axon_hooks = mod
        sys.modules["antenv.axon_hooks"] = mod
    try:
        from trn_agent_boot.trn_boot import _ntff_profile_via_ctypes
        from antenv.axon_hooks import set_axon_ntff_profile_hook
        set_axon_ntff_profile_hook(_ntff_profile_via_ctypes("/opt/axon/libaxon_pjrt.so"))
        bass_utils.upload_artifacts = lambda tmpdir: tmpdir
    except Exception:
        pass


def make_in_maps(cfg, weights, meta, streams):
    c = cfg
    indirect = cfg.get("GATHER") == "indirect"
    iota_rep = np.tile(np.tile(np.arange(128, dtype=np.float32), (128, 1)).astype(BF16),
                       (1, meta.NBmax))
    in_maps = []
    for cc in range(c["NCORES"]):
        in_maps.append(dict(
            xT=np.ascontiguousarray(weights["xT"][:, cc * c["SHARD"]:(cc + 1) * c["SHARD"]]),
            W1c=weights["W1cat"],
            W2c=weights["W2cat"],
            iota=weights["iota"],
            ident=weights["ident"],
            b1r=weights["b1rep"],
            b2r=weights["b2rep"],
            drel=streams[cc]["drel"],
            iotar=iota_rep,
            **({"offs": streams[cc]["offs"]} if indirect
               else {"idxs": streams[cc]["idx"]}),
        ))
    return in_maps


def run(cfg, inputs, trace=False):
    c = derive(cfg)
    x = np.asarray(inputs["x"])
    ei = np.asarray(inputs["edge_index"])
    weights = prep_weights(
        c, x, inputs["W1"], inputs["att_src1"], inputs["att_dst1"], inputs["b1"],
        inputs["W2"], inputs["att_src2"], inputs["att_dst2"], inputs["b2"])
    if c.get("SCHEME") == "pd64":
        meta, streams = prep_edges_pd(c, ei[0], ei[1])
        nc = build_kernel_pd(c, meta)
        in_maps = make_in_maps_pd(c, weights, meta, streams)
    else:
        meta, streams = prep_edges(c, ei[0], ei[1])
        nc = build_kernel(c, meta)
        in_maps = make_in_maps(c, weights, meta, streams)
    if trace:
        _install_ntff_shim()
    res = bass_utils.run_bass_kernel_spmd(
        nc, in_maps, core_ids=list(range(c["NCORES"])), trace=trace)
    outs = [res.results[cc]["out"] for cc in range(c["NCORES"])]
    full = np.concatenate(outs, axis=0)[: c["N"]]
    return full, res


def active_cfg():
    if os.environ.get("GAT_SCHEME") == "pd64":
        return pd_cfg()
    if os.environ.get("GAT_SCHEME") == "base":
        return full_cfg()
    return full_cfg()


def kernel(**inputs):
    out, _ = run(active_cfg(), inputs, trace=bool(os.environ.get("GAT_TRACE")))
    return out

